# revision 1
# baseline (speedup 1.0000x reference)
"""Multi-head attention (B=4, N=2048, DM=1024, H=16, DH=64) on 8 trn2 cores.

Sharding: core c -> (batch b = c//2, head-group hg = c%2 of 8 heads).
Each core computes qkv for its 8 heads, masked softmax attention, and a
partial output projection over its 512 head-dims.  Host sums the two
partials per batch and adds the bias.

Device-side layout ("feature-major"):
  - x^T [DM, N] so QK projection emits q^T/k^T [64, N] per head directly.
  - mask folded into q^T (x SCALE*m_i, SCALE pre-baked in w_q) and k^T
    (x m_j): masked score pairs become exp(0)=1; a rank-1 correction
    matmul (-m_i * C_h, with C_h = sum_{masked j} v_h[j,:]) cancels them
    for live queries, and dead queries (m_i=0) fall out as the exact
    uniform-softmax rows the reference produces.
  - v stored token-major with an appended ones column per head, so the
    PV matmul accumulates the softmax denominator for free.
  - per-head-pair QK projection is interleaved into the head loop so the
    PE keeps busy while ACT (exp) is the softmax bottleneck.
"""

import sys

sys.path.insert(0, "/opt/trn_rl_repo")

import numpy as np
import ml_dtypes

B, N, DM, H, DH = 4, 2048, 1024, 16, 64
SCALE = DH**-0.5
NCORES = 8
HG = 2  # head groups (tensor-parallel factor)
HL = H // HG  # 8 heads per core
NP = HL // 2  # 4 head pairs
FQK = HL * 2 * DH  # 1024 qk features per core
FV = HL * DH  # 512 v features per core
P = 128
NT = N // P  # 16 token tiles
DMT = DM // P  # 8 dm tiles
VW = DH + 1  # 65: v columns + ones column
VROW = HL * VW  # 520
HT = FV // P  # 4 head-dim tiles for the projection
NH = N // 2  # 1024: i-half width
NHT = NT // 2  # 8 token tiles per i-half

_CACHE = {}


def _build_program():
    import concourse.mybir as mybir
    import concourse.tile as tile
    from concourse import bacc
    from concourse.bass import ts
    from concourse.masks import make_identity

    bf = mybir.dt.bfloat16
    f32 = mybir.dt.float32
    EXP = mybir.ActivationFunctionType.Exp

    nc = bacc.Bacc(
        "TRN2", target_bir_lowering=False, debug=False, num_devices=NCORES
    )
    xT = nc.dram_tensor("xT", [DM, N], bf, kind="ExternalInput")
    wqk = nc.dram_tensor("wqk", [DM, FQK], bf, kind="ExternalInput")
    wv = nc.dram_tensor("wv", [DM, FV], bf, kind="ExternalInput")
    wout = nc.dram_tensor("wout", [FV, DM], bf, kind="ExternalInput")
    qkmask = nc.dram_tensor("qkmask", [P, N], f32, kind="ExternalInput")
    mrow = nc.dram_tensor("mrow", [1, N], bf, kind="ExternalInput")
    iminv = nc.dram_tensor("iminv", [P, NT], bf, kind="ExternalInput")
    out = nc.dram_tensor("out", [N, DM], f32, kind="ExternalOutput")

    with tile.TileContext(nc) as tc:
        with tc.tile_pool(name="const", bufs=1) as cp:
            xT_sb = cp.tile([P, DMT * N], bf, tag="xT")
            wqk_sb = cp.tile([P, DMT * FQK], bf, tag="wqk")
            wv_sb = cp.tile([P, DMT * FV], bf, tag="wv")
            wout_sb = cp.tile([P, HT * DM], bf, tag="wout")
            qkm_sb = cp.tile([P, N], f32, tag="qkm")
            mrow_sb = cp.tile([1, N], bf, tag="mrow")
            iminv_sb = cp.tile([P, NT], bf, tag="iminv")
            ident = cp.tile([P, P], bf, tag="ident")
            vplus = cp.tile([P, NT * VROW], bf, tag="vplus")
            qk_all = cp.tile([P, HL * N], bf, tag="qkall")
            attT = cp.tile([P, HT * N], bf, tag="attT")
            att_pair = cp.tile([P, NT * P], bf, tag="attpair")
            c_sb = cp.tile([1, VROW], bf, tag="csb")

            # DMA order mirrors consumption: the first QK chunk's inputs
            # (xT/wqk dmt 0) lead, then the small mask tensors (the DVE
            # program's first op waits on qkmask), then the remaining
            # xT/wqk tiles; wv and wout are needed later so they go last.
            nc.sync.dma_start(out=xT_sb[:, ts(0, N)], in_=xT[ts(0, P), :])
            nc.sync.dma_start(out=wqk_sb[:, ts(0, FQK)], in_=wqk[ts(0, P), :])
            nc.sync.dma_start(out=qkm_sb[:, :], in_=qkmask[:, :])
            nc.sync.dma_start(out=mrow_sb[:, :], in_=mrow[:, :])
            nc.sync.dma_start(out=iminv_sb[:, :], in_=iminv[:, :])
            for dmt in range(1, DMT):
                nc.sync.dma_start(out=xT_sb[:, ts(dmt, N)], in_=xT[ts(dmt, P), :])
                nc.sync.dma_start(out=wqk_sb[:, ts(dmt, FQK)], in_=wqk[ts(dmt, P), :])
            for dmt in range(DMT):
                nc.sync.dma_start(out=wv_sb[:, ts(dmt, FV)], in_=wv[ts(dmt, P), :])
            for ht in range(HT):
                nc.sync.dma_start(out=wout_sb[:, ts(ht, DM)], in_=wout[ts(ht, P), :])
            make_identity(nc, ident)

            vp4 = vplus.rearrange("p (t g c) -> p t g c", t=NT, g=HL, c=VW)
            nc.gpsimd.memset(vp4[:, :, :, DH], 1.0)

            # Prime the DVE vector clock on the mask DMA so the first
            # tensor_mul needs only the PE wait.
            scratch = cp.tile([1, 1], f32, tag="scratch")
            nc.vector.tensor_copy(scratch, qkm_sb[0:1, 0:1])

            with (
                tc.tile_pool(name="psqk", bufs=2, space="PSUM") as pqk,
                tc.tile_pool(name="pss", bufs=2, space="PSUM") as pss,
                tc.tile_pool(name="psa", bufs=1, space="PSUM") as psa,
                tc.tile_pool(name="tpool", bufs=27) as tp,
                tc.tile_pool(name="spool", bufs=4) as sp,
            ):

                def emit_qk_chunk(ft, qu):
                    ps_qk = pqk.tile([P, 512], f32, tag="qk", name="ps_qk")
                    for dmt in range(DMT):
                        nc.tensor.matmul(
                            ps_qk[:, :],
                            wqk_sb[:, dmt * FQK + ft * P : dmt * FQK + (ft + 1) * P],
                            xT_sb[:, dmt * N + qu * 512 : dmt * N + (qu + 1) * 512],
                            start=(dmt == 0),
                            stop=(dmt == DMT - 1),
                        )
                    nc.vector.tensor_mul(
                        qk_all[:, ft * N + qu * 512 : ft * N + (qu + 1) * 512],
                        ps_qk[:, :],
                        qkm_sb[:, ts(qu, 512)],
                    )

                def emit_qk(pair):
                    # q f-tile `pair` and k f-tile NP+pair, in N-quarters.
                    for ft in (pair, NP + pair):
                        for qu in range(4):
                            emit_qk_chunk(ft, qu)

                emit_qk(0)

                # Pre-emit head 0 / i-half 0 scores+exp ahead of the V
                # projection so ACT starts working ~16us in instead of
                # waiting for V+C (~60us).  The t tiles are consumed by the
                # regular PV loop below once vplus is ready.
                pre_t = []
                for ih in range(2):
                    for jt in range(NT):
                        t_sb = tp.tile([P, NH], bf, tag="t", name="t_sb")
                        kT = qk_all[0:64, NP * N + jt * P : NP * N + (jt + 1) * P]
                        ps_s = pss.tile([P, NH], f32, tag="s", name="ps_s")
                        for ch in range(2):
                            c0 = ih * NH + ch * 512
                            nc.tensor.matmul(
                                ps_s[:, ts(ch, 512)],
                                kT,
                                qk_all[0:64, c0 : c0 + 512],
                                start=True,
                                stop=True,
                            )
                        nc.scalar.activation(t_sb[:, :], ps_s[:, :], EXP)
                        pre_t.append(t_sb)

                # ---- V projection (token-major) + masked-v row C ----
                for tt in range(NT):
                    ps_v = pqk.tile([P, FV], f32, tag="qk", name="ps_v")
                    for dmt in range(DMT):
                        nc.tensor.matmul(
                            ps_v[:, :],
                            xT_sb[:, dmt * N + tt * P : dmt * N + (tt + 1) * P],
                            wv_sb[:, ts(dmt, FV)],
                            start=(dmt == 0),
                            stop=(dmt == DMT - 1),
                        )
                    nc.vector.tensor_copy(
                        vp4[:, tt, :, 0:DH],
                        ps_v.rearrange("p (g c) -> p g c", g=HL, c=DH),
                    )
                # C tiles live in the 1-bank "qk" slots: putting them in the
                # scores pool would pin both scores slots until the whole V
                # projection finishes, stalling ACT ~76us at startup.
                ps_c0 = pqk.tile([1, VROW // 2], f32, tag="qk", name="ps_c0")
                ps_c1 = pqk.tile([1, VROW // 2], f32, tag="qk", name="ps_c1")
                for jt in range(NT):
                    nc.tensor.matmul(
                        ps_c0[:, :],
                        iminv_sb[:, jt : jt + 1],
                        vplus[:, jt * VROW : jt * VROW + VROW // 2],
                        start=(jt == 0),
                        stop=(jt == NT - 1),
                    )
                    nc.tensor.matmul(
                        ps_c1[:, :],
                        iminv_sb[:, jt : jt + 1],
                        vplus[:, jt * VROW + VROW // 2 : (jt + 1) * VROW],
                        start=(jt == 0),
                        stop=(jt == NT - 1),
                    )
                nc.vector.tensor_scalar_mul(c_sb[:, 0 : VROW // 2], ps_c0[:, :], -1.0)
                nc.vector.tensor_scalar_mul(
                    c_sb[:, VROW // 2 : VROW], ps_c1[:, :], -1.0
                )

                # ---- head loop: scores^T -> exp -> PV -> normalize ----
                # Next pair's QK projection is spread 2 chunks per
                # (head, i-half) block so the PE fills its slack inside the
                # ACT-bound softmax phase instead of stalling ACT at pair
                # boundaries.
                for pair in range(NP):
                    next_chunks = (
                        [(ft, qu) for ft in (pair + 1, NP + pair + 1) for qu in range(4)]
                        if pair + 1 < NP
                        else []
                    )
                    blk = 0
                    for hh in range(2):
                        h = 2 * pair + hh
                        p0 = hh * 64
                        qcol = pair * N
                        kcol = (NP + pair) * N
                        for ih in range(2):
                            pa = psa.tile([P, NH], f32, tag="att", name="pa")
                            for jt in range(NT):
                                if h == 0:
                                    t_sb = pre_t[ih * NT + jt]
                                else:
                                    t_sb = tp.tile([P, NH], bf, tag="t", name="t_sb")
                                    kT = qk_all[
                                        p0 : p0 + 64,
                                        kcol + jt * P : kcol + (jt + 1) * P,
                                    ]
                                    ps_s = pss.tile([P, NH], f32, tag="s", name="ps_s")
                                    for ch in range(2):
                                        c0 = qcol + ih * NH + ch * 512
                                        nc.tensor.matmul(
                                            ps_s[:, ts(ch, 512)],
                                            kT,
                                            qk_all[p0 : p0 + 64, c0 : c0 + 512],
                                            start=True,
                                            stop=True,
                                        )
                                    nc.scalar.activation(t_sb[:, :], ps_s[:, :], EXP)
                                vslice = vplus[
                                    :, jt * VROW + h * VW : jt * VROW + (h + 1) * VW
                                ]
                                for it8 in range(NHT):
                                    nc.tensor.matmul(
                                        pa[:, it8 * P : it8 * P + VW],
                                        t_sb[:, ts(it8, P)],
                                        vslice,
                                        start=(jt == 0 and it8 % 4 == 0),
                                        stop=False,
                                    )
                            for it8 in range(NHT):
                                nc.tensor.matmul(
                                    pa[:, it8 * P : it8 * P + VW],
                                    mrow_sb[
                                        :, ih * NH + it8 * P : ih * NH + (it8 + 1) * P
                                    ],
                                    c_sb[:, h * VW : (h + 1) * VW],
                                    start=False,
                                    stop=(it8 % 4 == 3),
                                )
                            r_sb = sp.tile([P, NHT], f32, tag="r", name="r_sb")
                            pa3 = pa.rearrange("p (t c) -> p t c", t=NHT, c=P)
                            nc.vector.reciprocal(r_sb[:, :], pa3[:, :, DH])
                            for it8 in range(NHT):
                                it = ih * NHT + it8
                                dst = att_pair[
                                    :, it * P + p0 : it * P + p0 + DH
                                ]
                                nc.vector.tensor_scalar_mul(
                                    dst,
                                    pa[:, it8 * P : it8 * P + DH],
                                    r_sb[:, it8 : it8 + 1],
                                )
                            for ft_qu in next_chunks[2 * blk : 2 * blk + 2]:
                                emit_qk_chunk(*ft_qu)
                            blk += 1
                    for it in range(NT):
                        ps_tr = pqk.tile([P, P], bf, tag="qk", name="ps_tr")
                        nc.tensor.transpose(ps_tr[:, :], att_pair[:, ts(it, P)], ident)
                        nc.vector.tensor_copy(
                            attT[:, pair * N + it * P : pair * N + (it + 1) * P],
                            ps_tr[:, :],
                        )

                # ---- partial output projection ----
                # [128, 512] chunks so the tiles fit the 1-bank "qk" slots.
                # PSUM->SBUF eviction on the Scalar engine (idle here).
                COPY = mybir.ActivationFunctionType.Copy
                for it in range(NT):
                    for ch in range(2):
                        ps_o = pqk.tile([P, 512], f32, tag="qk", name="ps_o")
                        for ht in range(HT):
                            nc.tensor.matmul(
                                ps_o[:, :],
                                attT[:, ht * N + it * P : ht * N + (it + 1) * P],
                                wout_sb[
                                    :, ht * DM + ch * 512 : ht * DM + (ch + 1) * 512
                                ],
                                start=(ht == 0),
                                stop=(ht == HT - 1),
                            )
                        o_sb = sp.tile([P, 512], f32, tag="ob", name="o_sb")
                        # Alternate eviction engine so ACT and DVE each
                        # drain half the projection chunks in parallel.
                        if ch == 0:
                            nc.scalar.activation(o_sb[:, :], ps_o[:, :], COPY)
                        else:
                            nc.vector.tensor_copy(o_sb[:, :], ps_o[:, :])
                        nc.sync.dma_start(
                            out=out[ts(it, P), ts(ch, 512)], in_=o_sb[:, :]
                        )

    nc.compile()
    return nc


def _shard_inputs(x, w_qkv, w_out, b_out, mask):
    """Build the per-core input maps (host-side sharding + layout prep)."""
    bf = ml_dtypes.bfloat16
    x = np.asarray(x, dtype=np.float32)
    w_qkv = np.asarray(w_qkv, dtype=np.float32)
    w_out = np.asarray(w_out, dtype=np.float32)
    mask = np.asarray(mask)

    # w_qkv columns: head h occupies cols [h*192, (h+1)*192) as q|k|v of 64.
    w3 = w_qkv.reshape(DM, H, 3, DH)
    in_maps = []
    for c in range(NCORES):
        b, hg = c // HG, c % HG
        # q features for all 8 heads (cols 0:512, pre-scaled by SCALE),
        # then k features
        wqk_c = np.ascontiguousarray(
            np.concatenate(
                [
                    w3[:, hg * HL : (hg + 1) * HL, 0, :].reshape(DM, FV) * SCALE,
                    w3[:, hg * HL : (hg + 1) * HL, 1, :].reshape(DM, FV),
                ],
                axis=1,
            )
        ).astype(bf)
        wv_c = np.ascontiguousarray(
            w3[:, hg * HL : (hg + 1) * HL, 2, :].reshape(DM, FV)
        ).astype(bf)
        wout_c = np.ascontiguousarray(w_out[hg * FV : (hg + 1) * FV, :]).astype(bf)
        xT_c = np.ascontiguousarray(x[b].T).astype(bf)

        m = mask[b].astype(np.float32)  # [N] of 0/1
        qkm = np.broadcast_to(m[None, :], (P, N)).copy()
        mrow_c = m[None, :].astype(bf)
        iminv_c = np.ascontiguousarray((1.0 - m).reshape(NT, P).T).astype(bf)

        in_maps.append(
            {
                "xT": xT_c,
                "wqk": wqk_c,
                "wv": wv_c,
                "wout": wout_c,
                "qkmask": qkm,
                "mrow": mrow_c,
                "iminv": iminv_c,
            }
        )
    return in_maps


def kernel(x, w_qkv, w_out, b_out, mask):
    from concourse.bass_utils import run_bass_kernel_spmd

    if "nc" not in _CACHE:
        _CACHE["nc"] = _build_program()
    nc = _CACHE["nc"]

    in_maps = _shard_inputs(x, w_qkv, w_out, b_out, mask)
    res = run_bass_kernel_spmd(nc, in_maps, list(range(NCORES))).results

    b_out = np.asarray(b_out, dtype=np.float32)
    out = np.empty((B, N, DM), np.float32)
    for b in range(B):
        out[b] = res[HG * b]["out"] + res[HG * b + 1]["out"] + b_out[None, :]
    return out



# revision 32
# speedup vs baseline: 2.6473x; 2.6473x over previous
"""Multi-head attention (B=4, N=2048, DM=1024, H=16, DH=64) on 8 trn2 cores.

Sharding: core c -> (batch b = c//2, head-group hg = c%2 of 8 heads).

Live-token compaction: the pair mask m_i*m_j means masked keys contribute
exactly zero to every live query's softmax (exp(-1e6) == 0 in f32), and
masked queries get the uniform average of ALL values.  So:
  - host compacts each batch to its ~N/2 live tokens (padded to M, a
    multiple of 128), and the device runs plain UNMASKED attention on the
    compacted tokens: scores/exp/PV shrink ~(M/N)^2 ~ 3.2x, projections
    ~2x.  Padded tokens have x=0 and a zeroed "ones" column in v-plus, so
    they add exactly 0 to both the numerator and the softmax denominator.
  - dead-query rows (identical for every dead i within a batch: the
    uniform-softmax average of v over all 2048 tokens through the output
    projection) are computed exactly on the host in float64.

Device schedule (PE-bound; ACT exp is the secondary engine):
  - feature-major x^T so QK projection emits q^T/k^T [64, M] per head;
    SCALE pre-baked into w_q; v token-major with a live-flag column so PV
    accumulates the softmax denominator for free.
  - i-dim in two halves (ceil/floor of M/128 tiles); exp at [128, half]
    width on ACT.
  - scores+exp for block i+2 are emitted during block i (one-block
    lookahead) so PV never waits on ACT.
  - a filler FIFO streams pair-1..3 QK projection matmuls and the first
    half (ht0/ht1) of the output projection into the PE slack of the
    ACT-bound attention blocks; the final output projection only
    contracts ht2/ht3 and adds the SBUF partial back in.
  - input DMAs are split across the SP and ACT hardware queues with the
    pair-0 weight columns prioritized.
"""

import sys

sys.path.insert(0, "/opt/trn_rl_repo")

import numpy as np
import ml_dtypes

B, N, DM, H, DH = 4, 2048, 1024, 16, 64
SCALE = DH**-0.5
NCORES = 8
HG = 2  # head groups (tensor-parallel factor)
HL = H // HG  # 8 heads per core
NP = HL // 2  # 4 head pairs
FQK = HL * 2 * DH  # 1024 qk features per core
FV = HL * DH  # 512 v features per core
P = 128
DMT = DM // P  # 8 dm tiles
VW = DH + 1  # 65: v columns + live-flag column
HT = FV // P  # 4 head-dim tiles for the projection

_CACHE = {}


def _even_groups(total, maxw):
    """Split `total` into contiguous (offset, width) groups of width<=maxw."""
    n = -(-total // maxw)
    base, rem = divmod(total, n)
    out, off = [], 0
    for i in range(n):
        w = base + (1 if i < rem else 0)
        out.append((off, w))
        off += w
    return out


def _build_program(M, MEFF):
    import concourse.mybir as mybir
    import concourse.tile as tile
    from concourse import bacc
    from concourse.bass import ts
    from concourse.masks import make_identity

    bf = mybir.dt.bfloat16
    f32 = mybir.dt.float32
    EXP = mybir.ActivationFunctionType.Exp

    MT = M // P  # token tiles
    IHA = (MT + 1) // 2  # i-half A tiles
    IHB = MT - IHA  # i-half B tiles
    WA = IHA * P
    VROW = HL * VW
    groups = _even_groups(M, 512)  # token chunks for projections

    def _bank_chunks(total):
        """512-aligned (offset, width) chunks: matmul outputs must not
        cross a PSUM bank (512 f32) boundary."""
        out, off = [], 0
        while off < total:
            out.append((off, min(512, total - off)))
            off += 512
        return out

    # effective i-width: columns beyond the max live count only feed
    # output rows the host ignores (pad-key v rows are exact zeros either
    # way), so scores/exp skip them.  Scores for both i-halves live in one
    # [P, M] psum tile -> ONE exp per (head, j-tile).
    WEFFJ = max(32, min(M, MEFF))
    jchunks = _bank_chunks(WEFFJ)

    nc = bacc.Bacc(
        "TRN2", target_bir_lowering=False, debug=False, num_devices=NCORES
    )
    # All inputs are stored in SBUF-image layout [128, cols] (host
    # pre-swizzles) so each tensor loads with one (or a few) large DMAs:
    # the HWDGE descriptor generator is a serial ~630ns/DMA resource and
    # the DMA engines share one serial 360B/ns pipe, so count and order
    # are what matter.  wqk is split into the pair-0 columns (needed
    # first) and the rest.
    xT = nc.dram_tensor("xT", [P, DMT * M], bf, kind="ExternalInput")
    wqkp = nc.dram_tensor("wqkp", [P, DMT * 2 * P], bf, kind="ExternalInput")
    wqkr = nc.dram_tensor("wqkr", [P, DMT * 6 * P], bf, kind="ExternalInput")
    wv = nc.dram_tensor("wv", [P, DMT * FV], bf, kind="ExternalInput")
    wout = nc.dram_tensor("wout", [P, HT * DM], bf, kind="ExternalInput")
    vones = nc.dram_tensor("vones", [P, MT * HL], bf, kind="ExternalInput")
    out = nc.dram_tensor("out", [M, DM], bf, kind="ExternalOutput")

    with tile.TileContext(nc) as tc:
        with tc.tile_pool(name="const", bufs=1) as cp:
            xT_sb = cp.tile([P, DMT * M], bf, tag="xT")
            wqkp_sb = cp.tile([P, DMT * 2 * P], bf, tag="wqkp")
            wqkr_sb = cp.tile([P, DMT * 6 * P], bf, tag="wqkr")
            wv_sb = cp.tile([P, DMT * FV], bf, tag="wv")
            wout_sb = cp.tile([P, HT * DM], bf, tag="wout")
            vones_sb = cp.tile([P, MT * HL], bf, tag="vones")
            ident = cp.tile([P, P], bf, tag="ident")
            vplus = cp.tile([P, MT * VROW], bf, tag="vplus")
            qk_all = cp.tile([P, HL * M], bf, tag="qkall")
            attT = cp.tile([P, HT * M], bf, tag="attT")
            part01 = cp.tile([P, MT * DM], bf, tag="part01")

            # Single queue, consumption order; xT per-dmt so phase-1 QK
            # pipelines with the serial DMA stream.  wqkp as q-half then
            # k-half so the very first QK group starts ~0.8us earlier.
            wqkp4 = wqkp.rearrange("p (d two c) -> p d two c", d=DMT, two=2, c=P)
            wqkp4_sb = wqkp_sb.rearrange("p (d two c) -> p d two c", d=DMT, two=2, c=P)
            nc.sync.dma_start(out=wqkp4_sb[:, :, 0, :], in_=wqkp4[:, :, 0, :])
            nc.sync.dma_start(out=wqkp4_sb[:, :, 1, :], in_=wqkp4[:, :, 1, :])
            for dmt in range(DMT):
                nc.sync.dma_start(out=xT_sb[:, ts(dmt, M)], in_=xT[:, ts(dmt, M)])
            nc.sync.dma_start(out=wv_sb[:, :], in_=wv[:, :])
            nc.sync.dma_start(out=vones_sb[:, :], in_=vones[:, :])
            nc.sync.dma_start(out=wqkr_sb[:, :], in_=wqkr[:, :])
            nc.sync.dma_start(out=wout_sb[:, :], in_=wout[:, :])
            make_identity(nc, ident)

            def wqk_slice(ft, dmt):
                """Stationary [128, 128] weight tile for feature-tile ft."""
                if ft == 0:
                    return wqkp_sb[:, dmt * 2 * P : dmt * 2 * P + P]
                if ft == NP:
                    return wqkp_sb[:, dmt * 2 * P + P : dmt * 2 * P + 2 * P]
                ridx = ft - 1 if ft < NP else ft - 2
                return wqkr_sb[:, dmt * 6 * P + ridx * P : dmt * 6 * P + (ridx + 1) * P]

            vp4 = vplus.rearrange("p (t g c) -> p t g c", t=MT, g=HL, c=VW)
            TSLOTS = 20
            tstore = cp.tile([P, TSLOTS * M], bf, tag="tstore")
            tslot_ctr = [0]

            with (
                tc.tile_pool(name="psqk", bufs=3, space="PSUM") as pqk,
                tc.tile_pool(name="pss", bufs=1, space="PSUM") as pss,
                tc.tile_pool(name="psa", bufs=2, space="PSUM") as psa,
                tc.tile_pool(name="appool", bufs=2) as app,
                tc.tile_pool(name="spool", bufs=4) as sp,
            ):

                class Filler:
                    """FIFO of single-matmul-sized PE work units: pair-1..3
                    QK projection, then out-projection ht0/ht1 chunks."""

                    def __init__(self):
                        self.items = [
                            ("qk", ft, g0, gw, dmt)
                            for pair in range(1, NP)
                            for ft in (pair, NP + pair)
                            for (g0, gw) in groups
                            for dmt in range(DMT)
                        ] + [
                            ("opA", it, ch, s)
                            for it in range(MT)
                            for ch in range(2)
                            for s in range(2)
                        ]
                        self.pos = 0
                        self.ps = None
                        per_pair = 2 * len(groups) * DMT
                        self.qk_end = {
                            pair: (pair - 1 + 1) * per_pair for pair in range(1, NP)
                        }

                    def emit(self, n):
                        for _ in range(n):
                            if self.pos >= len(self.items):
                                return
                            item = self.items[self.pos]
                            if item[0] == "qk":
                                _, ft, g0, gw, dmt = item
                                if dmt == 0:
                                    self.ps = pqk.tile(
                                        [P, 512], f32, tag="qk", name="ps_qk"
                                    )
                                nc.tensor.matmul(
                                    self.ps[:, 0:gw],
                                    wqk_slice(ft, dmt),
                                    xT_sb[:, dmt * M + g0 : dmt * M + g0 + gw],
                                    start=(dmt == 0),
                                    stop=(dmt == DMT - 1),
                                )
                                if dmt == DMT - 1:
                                    nc.vector.tensor_copy(
                                        qk_all[:, ft * M + g0 : ft * M + g0 + gw],
                                        self.ps[:, 0:gw],
                                    )
                                    self.ps = None
                            elif item[0] == "opA":
                                _, it, ch, s = item
                                if s == 0:
                                    self.ps = pqk.tile(
                                        [P, 512], f32, tag="qk", name="ps_oa"
                                    )
                                nc.tensor.matmul(
                                    self.ps[:, :],
                                    attT[:, s * M + it * P : s * M + (it + 1) * P],
                                    wout_sb[
                                        :, s * DM + ch * 512 : s * DM + (ch + 1) * 512
                                    ],
                                    start=(s == 0),
                                    stop=(s == 1),
                                )
                                if s == 1:
                                    eng = nc.vector
                                    eng.tensor_copy(
                                        part01[
                                            :,
                                            it * DM + ch * 512 : it * DM + (ch + 1) * 512,
                                        ],
                                        self.ps[:, :],
                                    )
                                    self.ps = None
                            else:  # opA2: accumulate ht2 onto part01
                                _, it, ch, _ = item
                                ps = pqk.tile([P, 512], f32, tag="qk", name="ps_oa2")
                                nc.tensor.matmul(
                                    ps[:, :],
                                    attT[:, 2 * M + it * P : 2 * M + (it + 1) * P],
                                    wout_sb[
                                        :, 2 * DM + ch * 512 : 2 * DM + (ch + 1) * 512
                                    ],
                                    start=True,
                                    stop=True,
                                )
                                pslice = part01[
                                    :, it * DM + ch * 512 : it * DM + (ch + 1) * 512
                                ]
                                eng = nc.gpsimd if (it + ch) % 2 else nc.vector
                                eng.tensor_add(pslice, ps[:, :], pslice)
                            self.pos += 1

                    def drain_qk_pair(self, pair):
                        if pair in self.qk_end:
                            while self.pos < self.qk_end[pair]:
                                self.emit(1)

                # Pre-zero the t-ring columns an exp may leave unwritten
                # (beyond the effective i-width) so PV always reads
                # initialized data; runs on the otherwise-idle Pool engine
                # during the DMA-bound startup.
                if WEFFJ < M:
                    tst3 = tstore.rearrange("p (k w) -> p k w", k=TSLOTS, w=M)
                    nc.gpsimd.memset(tst3[:, :, WEFFJ:M], 1.0)

                # ---- phase 1: QK projection for pair 0 ----
                for ft in (0, NP):
                    for (g0, gw) in groups:
                        ps = pqk.tile([P, 512], f32, tag="qk", name="ps_qk")
                        for dmt in range(DMT):
                            nc.tensor.matmul(
                                ps[:, 0:gw],
                                wqk_slice(ft, dmt),
                                xT_sb[:, dmt * M + g0 : dmt * M + g0 + gw],
                                start=(dmt == 0),
                                stop=(dmt == DMT - 1),
                            )
                        nc.vector.tensor_copy(
                            qk_all[:, ft * M + g0 : ft * M + g0 + gw], ps[:, 0:gw]
                        )

                def score_exp(h, jt):
                    p0 = (h % 2) * 64
                    pair = h // 2
                    ps_s = pss.tile([P, M], f32, tag="s", name="ps_s")
                    kT = qk_all[
                        p0 : p0 + 64,
                        (NP + pair) * M + jt * P : (NP + pair) * M + (jt + 1) * P,
                    ]
                    for (off, w) in jchunks:
                        c0 = pair * M + off
                        nc.tensor.matmul(
                            ps_s[:, off : off + w],
                            kT,
                            qk_all[p0 : p0 + 64, c0 : c0 + w],
                            start=True,
                            stop=True,
                        )
                    slot = tslot_ctr[0] % TSLOTS
                    tslot_ctr[0] += 1
                    t = tstore[:, slot * M : (slot + 1) * M]
                    nc.scalar.activation(t[:, 0:WEFFJ], ps_s[:, 0:WEFFJ], EXP)
                    return t

                def pv(h, base, jt, t, pa, ntiles):
                    for it in range(ntiles):
                        nc.tensor.matmul(
                            pa[:, it * VW : (it + 1) * VW],
                            t[:, (base + it) * P : (base + it + 1) * P],
                            vplus[:, jt * VROW + h * VW : jt * VROW + (h + 1) * VW],
                            start=(jt == 0 and it == 0),
                            stop=(jt == MT - 1 and it == ntiles - 1),
                        )

                # ---- phase 2: head-0 scores/exp + V projection ----
                tmap = {}
                for jt in range(MT):
                    tmap[(0, jt)] = score_exp(0, jt)
                    ps_v = pqk.tile([P, 512], f32, tag="qk", name="ps_v")
                    for dmt in range(DMT):
                        nc.tensor.matmul(
                            ps_v[:, :],
                            xT_sb[:, dmt * M + jt * P : dmt * M + (jt + 1) * P],
                            wv_sb[:, ts(dmt, FV)],
                            start=(dmt == 0),
                            stop=(dmt == DMT - 1),
                        )
                    nc.vector.tensor_copy(
                        vp4[:, jt, :, 0:DH],
                        ps_v.rearrange("p (g c) -> p g c", g=HL, c=DH),
                    )
                    nc.gpsimd.tensor_copy(
                        vp4[:, jt, :, DH], vones_sb[:, jt * HL : (jt + 1) * HL]
                    )

                def transpose_half(pair, ap_tile, base, ntiles):
                    """Per-half transposes, batched 2 per psum tile, DVE
                    evictions (bf16 2x mode)."""
                    for it0 in range(base, base + ntiles, 2):
                        nb = min(2, base + ntiles - it0)
                        ps_tr = pqk.tile([P, 2 * P], bf, tag="qk", name="ps_tr")
                        for k in range(nb):
                            nc.tensor.transpose(
                                ps_tr[:, k * P : (k + 1) * P],
                                ap_tile[:, ts(it0 + k, P)],
                                ident,
                            )
                        nc.vector.tensor_copy(
                            attT[:, pair * M + it0 * P : pair * M + (it0 + nb) * P],
                            ps_tr[:, 0 : nb * P],
                        )

                def out_proj_b(its):
                    """Final output projection (ht3 + SBUF partial of
                    ht0..ht2) for the given i-tiles; psum alternates
                    pqk/pss rings, output DMAs alternate queues."""
                    for it in its:
                        o_sb = sp.tile([P, DM], bf, tag="ob", name="o_sb")
                        for ch in range(2):
                            if ch == 0:
                                ps_o = pqk.tile([P, 512], f32, tag="qk", name="ps_o")
                            else:
                                ps_o = pss.tile([P, M], f32, tag="s", name="ps_o")
                            for ht in (2, 3):
                                nc.tensor.matmul(
                                    ps_o[:, 0:512],
                                    attT[:, ht * M + it * P : ht * M + (it + 1) * P],
                                    wout_sb[
                                        :,
                                        ht * DM + ch * 512 : ht * DM + (ch + 1) * 512,
                                    ],
                                    start=(ht == 2),
                                    stop=False,
                                )
                            # identity matmul folds the ht0/ht1 SBUF partial
                            # into the psum accumulation on the PE
                            nc.tensor.matmul(
                                ps_o[:, 0:512],
                                ident,
                                part01[
                                    :, it * DM + ch * 512 : it * DM + (ch + 1) * 512
                                ],
                                start=False,
                                stop=True,
                            )
                            if ch == 0:
                                COPY = mybir.ActivationFunctionType.Copy
                                nc.scalar.activation(
                                    o_sb[:, ch * 512 : (ch + 1) * 512],
                                    ps_o[:, 0:512],
                                    COPY,
                                )
                            else:
                                nc.vector.tensor_copy(
                                    o_sb[:, ch * 512 : (ch + 1) * 512],
                                    ps_o[:, 0:512],
                                )
                        if it % 2 == 0:
                            nc.sync.dma_start(out=out[ts(it, P), :], in_=o_sb[:, :])
                        else:
                            nc.scalar.dma_start(out=out[ts(it, P), :], in_=o_sb[:, :])

                # ---- attention blocks with one-block score lookahead ----
                blocks = [(h, half) for h in range(HL) for half in (0, 1)]
                # filler budget per block: pair p+1's QK must complete
                # before block 2*(p+1) emits its lookahead scores; opA
                # (out-projection ht0/ht1) fills b10..b13.
                budgets = [24, 24] + [12] * 8 + [9] * 4 + [0, 0]
                filler = Filler()
                ap_tile = None
                for bi, (h, half) in enumerate(blocks):
                    pair = h // 2
                    p0 = (h % 2) * 64
                    if half == 0:
                        if h % 2 == 0:
                            ap_tile = app.tile([P, MT * P], bf, tag="ap", name="ap")
                    ntiles = IHA if half == 0 else IHB
                    base = 0 if half == 0 else IHA
                    pa = psa.tile([P, IHA * VW], f32, tag="att", name="pa")
                    pa3 = pa.rearrange("p (t c) -> p t c", t=IHA, c=VW)
                    budget = budgets[bi]
                    sched, acc = [], 0
                    for jt in range(MT):
                        acc += budget
                        sched.append(acc // MT - (acc - budget) // MT)
                    for jt in range(MT):
                        t = tmap[(h, jt)] if half == 0 else (
                            tmap.pop((h, jt)) if jt == MT - 1 or True else None
                        )
                        pv(h, base, jt, t, pa, ntiles)
                        if h + 1 < HL and jt % 2 == (0 if half == 0 else 1):
                            tmap[(h + 1, jt)] = score_exp(h + 1, jt)
                        filler.emit(sched[jt])
                    r = sp.tile([P, IHA], f32, tag="r", name="r_sb")
                    nc.vector.reciprocal(r[:, 0:ntiles], pa3[:, 0:ntiles, DH])
                    for it in range(ntiles):
                        eng = nc.vector
                        eng.tensor_scalar_mul(
                            ap_tile[
                                :, (base + it) * P + p0 : (base + it) * P + p0 + DH
                            ],
                            pa3[:, it, 0:DH],
                            r[:, it : it + 1],
                        )
                    if h % 2 == 1:
                        filler.drain_qk_pair(pair + 1)
                        transpose_half(pair, ap_tile, base, ntiles)
                        if pair == NP - 1:
                            filler.emit(len(filler.items))  # safety
                            out_proj_b(range(base, base + ntiles))

    nc.compile()
    return nc


def _compact(mask):
    """Live indices per batch and the padded capacity M (multiple of 128)."""
    live = [np.flatnonzero(np.asarray(mask[b]) != 0) for b in range(B)]
    mmax = max(1, max(len(l) for l in live))
    M = -(-mmax // P) * P
    return live, M


def _shard_inputs(x, w_qkv, w_out, b_out, mask):
    """Build the per-core input maps (host-side compaction + layout prep)."""
    bf = ml_dtypes.bfloat16
    x = np.asarray(x, dtype=np.float32)
    w_qkv = np.asarray(w_qkv, dtype=np.float32)
    w_out = np.asarray(w_out, dtype=np.float32)
    mask = np.asarray(mask)

    live, M = _compact(mask)
    MT = M // P

    def sbimg(a):
        """[DMT*P, C] -> SBUF image [P, DMT*C]."""
        r, ccols = a.shape
        d = r // P
        return np.ascontiguousarray(
            a.reshape(d, P, ccols).transpose(1, 0, 2).reshape(P, d * ccols)
        )

    # w_qkv columns: head h occupies cols [h*192, (h+1)*192) as q|k|v of 64.
    w3 = w_qkv.reshape(DM, H, 3, DH)
    in_maps = []
    for c in range(NCORES):
        b, hg = c // HG, c % HG
        idx = live[b]
        ml = len(idx)
        wq = w3[:, hg * HL : (hg + 1) * HL, 0, :].reshape(DM, FV) * SCALE
        wk = w3[:, hg * HL : (hg + 1) * HL, 1, :].reshape(DM, FV)
        # feature-tile ft of the q|k concat: q tiles 0..NP-1 hold head
        # pairs, k tiles NP..2NP-1 likewise.  Priority tensor = ft0|ft4
        # per dmt; rest tensor = ft 1,2,3,5,6,7 per dmt.
        fts = np.concatenate([wq, wk], axis=1).reshape(DM, 2 * NP, P)
        wqkp_c = sbimg(
            np.concatenate([fts[:, 0], fts[:, NP]], axis=1)
        ).astype(bf)
        rest_idx = [1, 2, 3, NP + 1, NP + 2, NP + 3]
        wqkr_c = sbimg(
            np.concatenate([fts[:, i] for i in rest_idx], axis=1)
        ).astype(bf)
        wv_c = sbimg(
            np.ascontiguousarray(
                w3[:, hg * HL : (hg + 1) * HL, 2, :].reshape(DM, FV)
            )
        ).astype(bf)
        wout_c = sbimg(
            np.ascontiguousarray(w_out[hg * FV : (hg + 1) * FV, :])
        ).astype(bf)
        xT_c = np.zeros((DM, M), dtype=np.float32)
        xT_c[:, :ml] = x[b][idx].T
        xT_c = sbimg(xT_c).astype(bf)
        ones = np.zeros((M,), np.float32)
        ones[:ml] = 1.0
        vones_c = np.ascontiguousarray(
            np.repeat(ones.reshape(MT, P).T[:, :, None], HL, axis=2).reshape(
                P, MT * HL
            )
        ).astype(bf)
        in_maps.append(
            {
                "xT": xT_c,
                "wqkp": wqkp_c,
                "wqkr": wqkr_c,
                "wv": wv_c,
                "wout": wout_c,
                "vones": vones_c,
            }
        )
    return in_maps, live, M


def kernel(x, w_qkv, w_out, b_out, mask):
    from concourse.bass_utils import run_bass_kernel_spmd

    x = np.asarray(x, dtype=np.float32)
    w_qkv = np.asarray(w_qkv, dtype=np.float32)
    w_out_f = np.asarray(w_out, dtype=np.float32)
    b_out = np.asarray(b_out, dtype=np.float32)
    mask = np.asarray(mask)

    in_maps, live, M = _shard_inputs(x, w_qkv, w_out_f, b_out, mask)
    meff = -(-max(1, max(len(l) for l in live)) // 32) * 32
    key = (M, meff)
    if key not in _CACHE:
        _CACHE[key] = _build_program(M, meff)
    nc = _CACHE[key]

    res = run_bass_kernel_spmd(nc, in_maps, list(range(NCORES))).results

    # dead-query rows: uniform softmax over ALL tokens -> mean of v rows,
    # through the output projection; exact in float64 on the host.
    w3 = w_qkv.astype(np.float64).reshape(DM, H, 3, DH)
    wv_full = w3[:, :, 2, :].reshape(DM, H * DH)
    out = np.empty((B, N, DM), np.float32)
    for b in range(B):
        idx = live[b]
        ml = len(idx)
        part = res[HG * b]["out"].astype(np.float32)[:ml] + res[HG * b + 1][
            "out"
        ].astype(np.float32)[:ml]
        dead_row = (
            x[b].astype(np.float64).mean(axis=0) @ wv_full @ w_out_f.astype(np.float64)
        ).astype(np.float32)
        out[b] = dead_row[None, :] + b_out[None, :]
        out[b][idx] = part + b_out[None, :]
    return out


# revision 40
# speedup vs baseline: 2.7081x; 1.0230x over previous
"""Multi-head attention (B=4, N=2048, DM=1024, H=16, DH=64) on 8 trn2 cores.

Sharding: core c -> (batch b = c//2, head-group hg = c%2 of 8 heads).

Live-token compaction: the pair mask m_i*m_j means masked keys contribute
exactly zero to every live query's softmax (exp(-1e6) == 0 in f32), and
masked queries get the uniform average of ALL values.  So:
  - host compacts each batch to its ~N/2 live tokens (padded to M, a
    multiple of 128), and the device runs plain UNMASKED attention on the
    compacted tokens: scores/exp/PV shrink ~(M/N)^2 ~ 3.2x, projections
    ~2x.  Padded tokens have x=0 and a zeroed "ones" column in v-plus, so
    they add exactly 0 to both the numerator and the softmax denominator.
  - dead-query rows (identical for every dead i within a batch: the
    uniform-softmax average of v over all 2048 tokens through the output
    projection) are computed exactly on the host in float64.

Device schedule (PE-bound; ACT exp is the secondary engine):
  - feature-major x^T so QK projection emits q^T/k^T [64, M] per head;
    SCALE pre-baked into w_q; v token-major with a live-flag column so PV
    accumulates the softmax denominator for free.
  - i-dim in two halves (ceil/floor of M/128 tiles); exp at [128, half]
    width on ACT.
  - scores+exp for block i+2 are emitted during block i (one-block
    lookahead) so PV never waits on ACT.
  - a filler FIFO streams pair-1..3 QK projection matmuls and the first
    half (ht0/ht1) of the output projection into the PE slack of the
    ACT-bound attention blocks; the final output projection only
    contracts ht2/ht3 and adds the SBUF partial back in.
  - input DMAs are split across the SP and ACT hardware queues with the
    pair-0 weight columns prioritized.
"""

import sys

sys.path.insert(0, "/opt/trn_rl_repo")

import numpy as np
import ml_dtypes

B, N, DM, H, DH = 4, 2048, 1024, 16, 64
SCALE = DH**-0.5
NCORES = 8
HG = 2  # head groups (tensor-parallel factor)
HL = H // HG  # 8 heads per core
NP = HL // 2  # 4 head pairs
FQK = HL * 2 * DH  # 1024 qk features per core
FV = HL * DH  # 512 v features per core
P = 128
DMT = DM // P  # 8 dm tiles
VW = DH + 1  # 65: v columns + live-flag column
HT = FV // P  # 4 head-dim tiles for the projection

_CACHE = {}


def _even_groups(total, maxw):
    """Split `total` into contiguous (offset, width) groups of width<=maxw."""
    n = -(-total // maxw)
    base, rem = divmod(total, n)
    out, off = [], 0
    for i in range(n):
        w = base + (1 if i < rem else 0)
        out.append((off, w))
        off += w
    return out


def _build_program(M, MEFF):
    import concourse.mybir as mybir
    import concourse.tile as tile
    from concourse import bacc
    from concourse.bass import ts
    from concourse.masks import make_identity

    bf = mybir.dt.bfloat16
    f32 = mybir.dt.float32
    EXP = mybir.ActivationFunctionType.Exp

    MT = M // P  # token tiles
    IHA = (MT + 1) // 2  # i-half A tiles
    IHB = MT - IHA  # i-half B tiles
    WA = IHA * P
    VROW = HL * VW
    groups = _even_groups(M, 512)  # token chunks for projections

    def _bank_chunks(total):
        """512-aligned (offset, width) chunks: matmul outputs must not
        cross a PSUM bank (512 f32) boundary."""
        out, off = [], 0
        while off < total:
            out.append((off, min(512, total - off)))
            off += 512
        return out

    # effective i-width per half: columns beyond the max live count
    # only feed output rows the host ignores (pad-key v rows are exact
    # zeros either way), so scores/exp skip them.
    WEFF = {
        0: max(32, min(IHA * P, MEFF)),
        1: max(32, min(IHB * P, MEFF - IHA * P)),
    }
    wchunks = {0: _bank_chunks(WEFF[0]), 1: _bank_chunks(WEFF[1])}
    # q-side projection columns beyond MEFF are never read by scores
    groups_q = _bank_chunks(min(M, max(512, MEFF)))

    nc = bacc.Bacc(
        "TRN2", target_bir_lowering=False, debug=False, num_devices=NCORES
    )
    # All inputs are stored in SBUF-image layout [128, cols] (host
    # pre-swizzles) so each tensor loads with one (or a few) large DMAs:
    # the HWDGE descriptor generator is a serial ~630ns/DMA resource and
    # the DMA engines share one serial 360B/ns pipe, so count and order
    # are what matter.  wqk is split into the pair-0 columns (needed
    # first) and the rest.
    xT = nc.dram_tensor("xT", [P, DMT * M], bf, kind="ExternalInput")
    wqkp = nc.dram_tensor("wqkp", [P, DMT * 2 * P], bf, kind="ExternalInput")
    wqkr = nc.dram_tensor("wqkr", [P, DMT * 6 * P], bf, kind="ExternalInput")
    wv = nc.dram_tensor("wv", [P, DMT * FV], bf, kind="ExternalInput")
    wout = nc.dram_tensor("wout", [P, HT * DM], bf, kind="ExternalInput")
    vones = nc.dram_tensor("vones", [P, MT * HL], bf, kind="ExternalInput")
    out = nc.dram_tensor("out", [M, DM], bf, kind="ExternalOutput")

    with tile.TileContext(nc) as tc:
        with tc.tile_pool(name="const", bufs=1) as cp:
            xT_sb = cp.tile([P, DMT * M], bf, tag="xT")
            wqkp_sb = cp.tile([P, DMT * 2 * P], bf, tag="wqkp")
            wqkr_sb = cp.tile([P, DMT * 6 * P], bf, tag="wqkr")
            wv_sb = cp.tile([P, DMT * FV], bf, tag="wv")
            wout_sb = cp.tile([P, HT * DM], bf, tag="wout")
            vones_sb = cp.tile([P, MT * HL], bf, tag="vones")
            ident = cp.tile([P, P], bf, tag="ident")
            vplus = cp.tile([P, MT * VROW], bf, tag="vplus")
            qk_all = cp.tile([P, HL * M], bf, tag="qkall")
            attT = cp.tile([P, HT * M], bf, tag="attT")
            part01 = cp.tile([P, MT * DM], bf, tag="part01")

            # Single queue, consumption order; xT per-dmt so phase-1 QK
            # pipelines with the serial DMA stream.  wqkp as q-half then
            # k-half so the very first QK group starts ~0.8us earlier.
            nc.sync.dma_start(out=wqkp_sb[:, :], in_=wqkp[:, :])
            for dmt in range(DMT):
                nc.sync.dma_start(out=xT_sb[:, ts(dmt, M)], in_=xT[:, ts(dmt, M)])
            nc.sync.dma_start(out=wv_sb[:, :], in_=wv[:, :])
            nc.sync.dma_start(out=vones_sb[:, :], in_=vones[:, :])
            nc.sync.dma_start(out=wqkr_sb[:, :], in_=wqkr[:, :])
            nc.sync.dma_start(out=wout_sb[:, :], in_=wout[:, :])
            make_identity(nc, ident)

            def wqk_slice(ft, dmt):
                """Stationary [128, 128] weight tile for feature-tile ft."""
                if ft == 0:
                    return wqkp_sb[:, dmt * 2 * P : dmt * 2 * P + P]
                if ft == NP:
                    return wqkp_sb[:, dmt * 2 * P + P : dmt * 2 * P + 2 * P]
                ridx = ft - 1 if ft < NP else ft - 2
                return wqkr_sb[:, dmt * 6 * P + ridx * P : dmt * 6 * P + (ridx + 1) * P]

            vp4 = vplus.rearrange("p (t g c) -> p t g c", t=MT, g=HL, c=VW)
            TSLOTS = 28
            tstore = cp.tile([P, TSLOTS * WA], bf, tag="tstore")
            tslot_ctr = [0]

            with (
                tc.tile_pool(name="psqk", bufs=2, space="PSUM") as pqk,
                tc.tile_pool(name="pss", bufs=2, space="PSUM") as pss,
                tc.tile_pool(name="psa", bufs=2, space="PSUM") as psa,
                tc.tile_pool(name="appool", bufs=2) as app,
                tc.tile_pool(name="spool", bufs=4) as sp,
            ):

                class Filler:
                    """FIFO of single-matmul-sized PE work units: pair-1..3
                    QK projection, then out-projection ht0/ht1 chunks."""

                    def __init__(self):
                        self.items = [
                            ("qk", ft, g0, gw, dmt)
                            for pair in range(1, NP)
                            for ft in (pair, NP + pair)
                            for (g0, gw) in (groups_q if ft < NP else groups)
                            for dmt in range(DMT)
                        ] + [
                            ("opA", it, ch, s)
                            for it in range(MT)
                            for ch in range(2)
                            for s in range(2)
                        ]
                        self.pos = 0
                        self.ps = None
                        per_pair = (len(groups_q) + len(groups)) * DMT
                        self.qk_end = {
                            pair: pair * per_pair for pair in range(1, NP)
                        }

                    def emit(self, n):
                        for _ in range(n):
                            if self.pos >= len(self.items):
                                return
                            item = self.items[self.pos]
                            if item[0] == "qk":
                                _, ft, g0, gw, dmt = item
                                if dmt == 0:
                                    self.ps = pqk.tile(
                                        [P, 512], f32, tag="qk", name="ps_qk"
                                    )
                                nc.tensor.matmul(
                                    self.ps[:, 0:gw],
                                    wqk_slice(ft, dmt),
                                    xT_sb[:, dmt * M + g0 : dmt * M + g0 + gw],
                                    start=(dmt == 0),
                                    stop=(dmt == DMT - 1),
                                )
                                if dmt == DMT - 1:
                                    nc.vector.tensor_copy(
                                        qk_all[:, ft * M + g0 : ft * M + g0 + gw],
                                        self.ps[:, 0:gw],
                                    )
                                    self.ps = None
                            elif item[0] == "opA":
                                _, it, ch, s = item
                                if s == 0:
                                    self.ps = pqk.tile(
                                        [P, 512], f32, tag="qk", name="ps_oa"
                                    )
                                nc.tensor.matmul(
                                    self.ps[:, :],
                                    attT[:, s * M + it * P : s * M + (it + 1) * P],
                                    wout_sb[
                                        :, s * DM + ch * 512 : s * DM + (ch + 1) * 512
                                    ],
                                    start=(s == 0),
                                    stop=(s == 1),
                                )
                                if s == 1:
                                    eng = nc.vector
                                    eng.tensor_copy(
                                        part01[
                                            :,
                                            it * DM + ch * 512 : it * DM + (ch + 1) * 512,
                                        ],
                                        self.ps[:, :],
                                    )
                                    self.ps = None
                            else:  # opA2: accumulate ht2 onto part01
                                _, it, ch, _ = item
                                ps = pqk.tile([P, 512], f32, tag="qk", name="ps_oa2")
                                nc.tensor.matmul(
                                    ps[:, :],
                                    attT[:, 2 * M + it * P : 2 * M + (it + 1) * P],
                                    wout_sb[
                                        :, 2 * DM + ch * 512 : 2 * DM + (ch + 1) * 512
                                    ],
                                    start=True,
                                    stop=True,
                                )
                                pslice = part01[
                                    :, it * DM + ch * 512 : it * DM + (ch + 1) * 512
                                ]
                                eng = nc.gpsimd if (it + ch) % 2 else nc.vector
                                eng.tensor_add(pslice, ps[:, :], pslice)
                            self.pos += 1

                    def drain_qk_pair(self, pair):
                        if pair in self.qk_end:
                            while self.pos < self.qk_end[pair]:
                                self.emit(1)

                # Pre-zero the t-ring columns an exp may leave unwritten
                # (beyond the effective i-width) so PV always reads
                # initialized data; runs on the otherwise-idle Pool engine
                # during the DMA-bound startup.
                wmin = min(WEFF.values())
                if wmin < WA:
                    tst3 = tstore.rearrange("p (k w) -> p k w", k=TSLOTS, w=WA)
                    nc.gpsimd.memset(tst3[:, :, wmin:WA], 1.0)

                # ---- phase 1: QK projection for pair 0 ----
                # dmt-outer over 4 concurrent psum groups (pss slots are
                # idle here) so the PE consumes each xT tile as the serial
                # DMA stream delivers it; remaining groups in a quick
                # second pass once everything is resident.
                p1a = [(0, groups_q[0]), (0, groups_q[1]), (NP, groups[0]),
                       (NP, groups[1])]
                p1b = [(0, g) for g in groups_q[2:]] + [
                    (NP, g) for g in groups[2:]
                ]
                ps_map = {}
                for i, (ft, (g0, gw)) in enumerate(p1a):
                    pool = pqk if i < 2 else pss
                    shape = [P, 512] if i < 2 else [P, WA]
                    tag = "qk" if i < 2 else "s"
                    ps_map[(ft, g0)] = pool.tile(shape, f32, tag=tag, name="ps_p1")
                for dmt in range(DMT):
                    for (ft, (g0, gw)) in p1a:
                        nc.tensor.matmul(
                            ps_map[(ft, g0)][:, 0:gw],
                            wqk_slice(ft, dmt),
                            xT_sb[:, dmt * M + g0 : dmt * M + g0 + gw],
                            start=(dmt == 0),
                            stop=(dmt == DMT - 1),
                        )
                for (ft, (g0, gw)) in p1a:
                    nc.vector.tensor_copy(
                        qk_all[:, ft * M + g0 : ft * M + g0 + gw],
                        ps_map[(ft, g0)][:, 0:gw],
                    )
                for (ft, (g0, gw)) in p1b:
                    ps = pqk.tile([P, 512], f32, tag="qk", name="ps_qk")
                    for dmt in range(DMT):
                        nc.tensor.matmul(
                            ps[:, 0:gw],
                            wqk_slice(ft, dmt),
                            xT_sb[:, dmt * M + g0 : dmt * M + g0 + gw],
                            start=(dmt == 0),
                            stop=(dmt == DMT - 1),
                        )
                    nc.vector.tensor_copy(
                        qk_all[:, ft * M + g0 : ft * M + g0 + gw], ps[:, 0:gw]
                    )

                def score_exp(h, half, jt):
                    p0 = (h % 2) * 64
                    pair = h // 2
                    base = 0 if half == 0 else IHA
                    W = WEFF[half]
                    ps_s = pss.tile([P, WA], f32, tag="s", name="ps_s")
                    kT = qk_all[
                        p0 : p0 + 64,
                        (NP + pair) * M + jt * P : (NP + pair) * M + (jt + 1) * P,
                    ]
                    for (off, w) in wchunks[half]:
                        c0 = pair * M + base * P + off
                        nc.tensor.matmul(
                            ps_s[:, off : off + w],
                            kT,
                            qk_all[p0 : p0 + 64, c0 : c0 + w],
                            start=True,
                            stop=True,
                        )
                    slot = tslot_ctr[0] % TSLOTS
                    tslot_ctr[0] += 1
                    t = tstore[:, slot * WA : (slot + 1) * WA]
                    nc.scalar.activation(t[:, 0:W], ps_s[:, 0:W], EXP)
                    return t

                def pv(h, half, jt, t, pa, ntiles):
                    for it in range(ntiles):
                        nc.tensor.matmul(
                            pa[:, it * VW : (it + 1) * VW],
                            t[:, ts(it, P)],
                            vplus[:, jt * VROW + h * VW : jt * VROW + (h + 1) * VW],
                            start=(jt == 0 and it == 0),
                            stop=(jt == MT - 1 and it == ntiles - 1),
                        )

                # ---- phase 2: head-0 scores/exp + V projection ----
                tmap = {}
                for jt in range(MT):
                    tmap[(0, 0, jt)] = score_exp(0, 0, jt)
                    tmap[(0, 1, jt)] = score_exp(0, 1, jt)
                    ps_v = pqk.tile([P, 512], f32, tag="qk", name="ps_v")
                    for dmt in range(DMT):
                        nc.tensor.matmul(
                            ps_v[:, :],
                            xT_sb[:, dmt * M + jt * P : dmt * M + (jt + 1) * P],
                            wv_sb[:, ts(dmt, FV)],
                            start=(dmt == 0),
                            stop=(dmt == DMT - 1),
                        )
                    nc.vector.tensor_copy(
                        vp4[:, jt, :, 0:DH],
                        ps_v.rearrange("p (g c) -> p g c", g=HL, c=DH),
                    )
                    nc.gpsimd.tensor_copy(
                        vp4[:, jt, :, DH], vones_sb[:, jt * HL : (jt + 1) * HL]
                    )

                def transpose_half(pair, ap_tile, base, ntiles):
                    """Per-half transposes, batched 2 per psum tile, DVE
                    evictions (bf16 2x mode)."""
                    for it0 in range(base, base + ntiles, 2):
                        nb = min(2, base + ntiles - it0)
                        ps_tr = pqk.tile([P, 2 * P], bf, tag="qk", name="ps_tr")
                        for k in range(nb):
                            nc.tensor.transpose(
                                ps_tr[:, k * P : (k + 1) * P],
                                ap_tile[:, ts(it0 + k, P)],
                                ident,
                            )
                        nc.vector.tensor_copy(
                            attT[:, pair * M + it0 * P : pair * M + (it0 + nb) * P],
                            ps_tr[:, 0 : nb * P],
                        )

                def out_proj_b(its):
                    """Final output projection (ht3 + SBUF partial of
                    ht0..ht2) for the given i-tiles; psum alternates
                    pqk/pss rings, output DMAs alternate queues."""
                    for it in its:
                        o_sb = sp.tile([P, DM], bf, tag="ob", name="o_sb")
                        for ch in range(2):
                            if ch == 0:
                                ps_o = pqk.tile([P, 512], f32, tag="qk", name="ps_o")
                            else:
                                ps_o = pss.tile([P, WA], f32, tag="s", name="ps_o")
                            for ht in (2, 3):
                                nc.tensor.matmul(
                                    ps_o[:, 0:512],
                                    attT[:, ht * M + it * P : ht * M + (it + 1) * P],
                                    wout_sb[
                                        :,
                                        ht * DM + ch * 512 : ht * DM + (ch + 1) * 512,
                                    ],
                                    start=(ht == 2),
                                    stop=(ht == 3 and ch == 1),
                                )
                            if ch == 0:
                                # identity matmul folds the ht0/ht1 SBUF
                                # partial into the psum accumulation on PE,
                                # then ACT (idle here) evicts with a copy
                                nc.tensor.matmul(
                                    ps_o[:, 0:512],
                                    ident,
                                    part01[:, it * DM : it * DM + 512],
                                    start=False,
                                    stop=True,
                                )
                                COPY = mybir.ActivationFunctionType.Copy
                                nc.scalar.activation(
                                    o_sb[:, 0:512], ps_o[:, 0:512], COPY
                                )
                                nc.sync.dma_start(
                                    out=out[ts(it, P), 0:512], in_=o_sb[:, 0:512]
                                )
                            else:
                                nc.vector.tensor_add(
                                    o_sb[:, 512:DM],
                                    ps_o[:, 0:512],
                                    part01[:, it * DM + 512 : (it + 1) * DM],
                                )
                                nc.scalar.dma_start(
                                    out=out[ts(it, P), 512:DM], in_=o_sb[:, 512:DM]
                                )

                # ---- attention blocks with one-block score lookahead ----
                blocks = [(h, half) for h in range(HL) for half in (0, 1)]
                # filler budget per block: pair p+1's QK must complete
                # before block 2*(p+1) emits its lookahead scores; opA
                # (out-projection ht0/ht1) fills b10..b13.
                budgets = [24, 24] + [12] * 8 + [9] * 4 + [0, 0]
                filler = Filler()
                ap_tile = None
                for bi, (h, half) in enumerate(blocks):
                    pair = h // 2
                    p0 = (h % 2) * 64
                    if half == 0:
                        if h % 2 == 0:
                            ap_tile = app.tile([P, MT * P], bf, tag="ap", name="ap")
                    ntiles = IHA if half == 0 else IHB
                    base = 0 if half == 0 else IHA
                    pa = psa.tile([P, IHA * VW], f32, tag="att", name="pa")
                    pa3 = pa.rearrange("p (t c) -> p t c", t=IHA, c=VW)
                    budget = budgets[bi]
                    sched, acc = [], 0
                    for jt in range(MT):
                        acc += budget
                        sched.append(acc // MT - (acc - budget) // MT)
                    for jt in range(MT):
                        pv(h, half, jt, tmap.pop((h, half, jt)), pa, ntiles)
                        if bi + 2 < len(blocks):
                            h2, half2 = blocks[bi + 2]
                            tmap[(h2, half2, jt)] = score_exp(h2, half2, jt)
                        filler.emit(sched[jt])
                    r = sp.tile([P, IHA], f32, tag="r", name="r_sb")
                    nc.vector.reciprocal(r[:, 0:ntiles], pa3[:, 0:ntiles, DH])
                    for it in range(ntiles):
                        eng = nc.vector
                        eng.tensor_scalar_mul(
                            ap_tile[
                                :, (base + it) * P + p0 : (base + it) * P + p0 + DH
                            ],
                            pa3[:, it, 0:DH],
                            r[:, it : it + 1],
                        )
                    if h % 2 == 1:
                        filler.drain_qk_pair(pair + 1)
                        transpose_half(pair, ap_tile, base, ntiles)
                        if pair == NP - 1:
                            filler.emit(len(filler.items))  # safety
                            out_proj_b(range(base, base + ntiles))

    nc.compile()
    return nc


def _compact(mask):
    """Live indices per batch and the padded capacity M (multiple of 128)."""
    live = [np.flatnonzero(np.asarray(mask[b]) != 0) for b in range(B)]
    mmax = max(1, max(len(l) for l in live))
    M = -(-mmax // P) * P
    return live, M


def _shard_inputs(x, w_qkv, w_out, b_out, mask):
    """Build the per-core input maps (host-side compaction + layout prep)."""
    bf = ml_dtypes.bfloat16
    x = np.asarray(x, dtype=np.float32)
    w_qkv = np.asarray(w_qkv, dtype=np.float32)
    w_out = np.asarray(w_out, dtype=np.float32)
    mask = np.asarray(mask)

    live, M = _compact(mask)
    MT = M // P

    def sbimg(a):
        """[DMT*P, C] -> SBUF image [P, DMT*C]."""
        r, ccols = a.shape
        d = r // P
        return np.ascontiguousarray(
            a.reshape(d, P, ccols).transpose(1, 0, 2).reshape(P, d * ccols)
        )

    # w_qkv columns: head h occupies cols [h*192, (h+1)*192) as q|k|v of 64.
    w3 = w_qkv.reshape(DM, H, 3, DH)
    in_maps = []
    for c in range(NCORES):
        b, hg = c // HG, c % HG
        idx = live[b]
        ml = len(idx)
        wq = w3[:, hg * HL : (hg + 1) * HL, 0, :].reshape(DM, FV) * SCALE
        wk = w3[:, hg * HL : (hg + 1) * HL, 1, :].reshape(DM, FV)
        # feature-tile ft of the q|k concat: q tiles 0..NP-1 hold head
        # pairs, k tiles NP..2NP-1 likewise.  Priority tensor = ft0|ft4
        # per dmt; rest tensor = ft 1,2,3,5,6,7 per dmt.
        fts = np.concatenate([wq, wk], axis=1).reshape(DM, 2 * NP, P)
        wqkp_c = sbimg(
            np.concatenate([fts[:, 0], fts[:, NP]], axis=1)
        ).astype(bf)
        rest_idx = [1, 2, 3, NP + 1, NP + 2, NP + 3]
        wqkr_c = sbimg(
            np.concatenate([fts[:, i] for i in rest_idx], axis=1)
        ).astype(bf)
        wv_c = sbimg(
            np.ascontiguousarray(
                w3[:, hg * HL : (hg + 1) * HL, 2, :].reshape(DM, FV)
            )
        ).astype(bf)
        wout_c = sbimg(
            np.ascontiguousarray(w_out[hg * FV : (hg + 1) * FV, :])
        ).astype(bf)
        xT_c = np.zeros((DM, M), dtype=np.float32)
        xT_c[:, :ml] = x[b][idx].T
        xT_c = sbimg(xT_c).astype(bf)
        ones = np.zeros((M,), np.float32)
        ones[:ml] = 1.0
        vones_c = np.ascontiguousarray(
            np.repeat(ones.reshape(MT, P).T[:, :, None], HL, axis=2).reshape(
                P, MT * HL
            )
        ).astype(bf)
        in_maps.append(
            {
                "xT": xT_c,
                "wqkp": wqkp_c,
                "wqkr": wqkr_c,
                "wv": wv_c,
                "wout": wout_c,
                "vones": vones_c,
            }
        )
    return in_maps, live, M


def kernel(x, w_qkv, w_out, b_out, mask):
    from concourse.bass_utils import run_bass_kernel_spmd

    x = np.asarray(x, dtype=np.float32)
    w_qkv = np.asarray(w_qkv, dtype=np.float32)
    w_out_f = np.asarray(w_out, dtype=np.float32)
    b_out = np.asarray(b_out, dtype=np.float32)
    mask = np.asarray(mask)

    in_maps, live, M = _shard_inputs(x, w_qkv, w_out_f, b_out, mask)
    meff = -(-max(1, max(len(l) for l in live)) // 32) * 32
    key = (M, meff)
    if key not in _CACHE:
        _CACHE[key] = _build_program(M, meff)
    nc = _CACHE[key]

    res = run_bass_kernel_spmd(nc, in_maps, list(range(NCORES))).results

    # dead-query rows: uniform softmax over ALL tokens -> mean of v rows,
    # through the output projection; exact in float64 on the host.
    w3 = w_qkv.astype(np.float64).reshape(DM, H, 3, DH)
    wv_full = w3[:, :, 2, :].reshape(DM, H * DH)
    out = np.empty((B, N, DM), np.float32)
    for b in range(B):
        idx = live[b]
        ml = len(idx)
        part = res[HG * b]["out"].astype(np.float32)[:ml] + res[HG * b + 1][
            "out"
        ].astype(np.float32)[:ml]
        dead_row = (
            x[b].astype(np.float64).mean(axis=0) @ wv_full @ w_out_f.astype(np.float64)
        ).astype(np.float32)
        out[b] = dead_row[None, :] + b_out[None, :]
        out[b][idx] = part + b_out[None, :]
    return out


# revision 80
# speedup vs baseline: 3.3518x; 1.2377x over previous
"""Multi-head attention (B=4, N=2048, DM=1024, H=16, DH=64) on 8 trn2 cores.

Sharding: core c -> (batch b = c//2, head-group hg = c%2 of 8 heads).

Live-token compaction: the pair mask m_i*m_j means masked keys contribute
exactly zero to every live query's softmax (exp(-1e6) == 0 in f32), and
masked queries get the uniform average of ALL values.  So:
  - host compacts each batch to its ~N/2 live tokens (padded to M, a
    multiple of 128), and the device runs plain UNMASKED attention on the
    compacted tokens: scores/exp/PV shrink ~(M/N)^2 ~ 3.2x, projections
    ~2x.  Padded tokens have x=0 and a zeroed "ones" column in v-plus, so
    they add exactly 0 to both the numerator and the softmax denominator.
  - dead-query rows (identical for every dead i within a batch: the
    uniform-softmax average of v over all 2048 tokens through the output
    projection) are computed exactly on the host in float64.

Device schedule (PE-bound; ACT exp is the secondary engine):
  - feature-major x^T so QK projection emits q^T/k^T [64, M] per head;
    SCALE pre-baked into w_q; v token-major with a live-flag column so PV
    accumulates the softmax denominator for free.
  - i-dim in two halves (ceil/floor of M/128 tiles); exp at [128, half]
    width on ACT.
  - scores+exp for block i+2 are emitted during block i (one-block
    lookahead) so PV never waits on ACT.
  - a filler FIFO streams pair-1..3 QK projection matmuls and the first
    half (ht0/ht1) of the output projection into the PE slack of the
    ACT-bound attention blocks; the final output projection only
    contracts ht2/ht3 and adds the SBUF partial back in.
  - inputs are stored in SBUF-image layout and loaded with a few large
    DMAs in consumption order (HWDGE descriptor generation is a serial
    ~630ns/DMA resource; DMA engines share one serial ~360B/ns pipe).
"""

import sys

sys.path.insert(0, "/opt/trn_rl_repo")

import numpy as np
import ml_dtypes

B, N, DM, H, DH = 4, 2048, 1024, 16, 64
SCALE = DH**-0.5
NCORES = 8
HG = 2  # head groups (tensor-parallel factor)
HL = H // HG  # 8 heads per core
NP = HL // 2  # 4 head pairs
FQK = HL * 2 * DH  # 1024 qk features per core
FV = HL * DH  # 512 v features per core
P = 128
DMT = DM // P  # 8 dm tiles
VW = DH + 1  # 65: v columns + live-flag column
HT = FV // P  # 4 head-dim tiles for the projection

_CACHE = {}


def _even_groups(total, maxw):
    """Split `total` into contiguous (offset, width) groups of width<=maxw."""
    n = -(-total // maxw)
    base, rem = divmod(total, n)
    out, off = [], 0
    for i in range(n):
        w = base + (1 if i < rem else 0)
        out.append((off, w))
        off += w
    return out


def _build_program(M, MEFF):
    import concourse.mybir as mybir
    import concourse.tile as tile
    from concourse import bacc
    from concourse.bass import ts
    from concourse.masks import make_identity

    bf = mybir.dt.bfloat16
    f32 = mybir.dt.float32
    EXP = mybir.ActivationFunctionType.Exp

    MT = M // P  # token (key) tiles
    VROW = HL * VW
    groups = _even_groups(M, 512)  # token chunks for projections

    def _bank_chunks(total):
        """512-aligned (offset, width) chunks: matmul outputs must not
        cross a PSUM bank (512 f32) boundary."""
        out, off = [], 0
        while off < total:
            out.append((off, min(512, total - off)))
            off += 512
        return out

    # Device computes queries for the first IEFF=min(M,1024) tokens only
    # (overflow live queries are handled on the host from exported k/v).
    # This makes the joint scores psum tile [P, IEFF] exactly 2 banks, so
    # ONE exp per (head, j-tile) instead of two.
    IT = min(MT, 8)  # device i-tiles
    IEFF = IT * P
    IHA = (IT + 1) // 2
    IHB = IT - IHA
    WEFFJ = max(32, min(IEFF, MEFF))  # trimmed exp width
    jchunks = _bank_chunks(WEFFJ)
    # q-side projection columns beyond WEFFJ are never read by scores
    groups_q = _bank_chunks(min(M, max(512, WEFFJ)))

    nc = bacc.Bacc(
        "TRN2", target_bir_lowering=False, debug=False, num_devices=NCORES
    )
    # All inputs are stored in SBUF-image layout [128, cols] (host
    # pre-swizzles) so each tensor loads with one (or a few) large DMAs:
    # the HWDGE descriptor generator is a serial ~630ns/DMA resource and
    # the DMA engines share one serial 360B/ns pipe, so count and order
    # are what matter.  wqk is split into the pair-0 columns (needed
    # first) and the rest.
    xT = nc.dram_tensor("xT", [P, DMT * M], bf, kind="ExternalInput")
    wqkpq = nc.dram_tensor("wqkpq", [P, DMT * P], bf, kind="ExternalInput")
    wqkpk = nc.dram_tensor("wqkpk", [P, DMT * P], bf, kind="ExternalInput")
    wqkr = nc.dram_tensor("wqkr", [P, DMT * 6 * P], bf, kind="ExternalInput")
    wv = nc.dram_tensor("wv", [P, DMT * FV], bf, kind="ExternalInput")
    wout = nc.dram_tensor("wout", [P, HT * DM], bf, kind="ExternalInput")
    vones = nc.dram_tensor("vones", [P, MT * HL], bf, kind="ExternalInput")
    out = nc.dram_tensor("out", [IEFF, DM], bf, kind="ExternalOutput")
    kx = nc.dram_tensor("kx", [P, NP * M], bf, kind="ExternalOutput")
    vpx = nc.dram_tensor("vpx", [P, MT * VROW], bf, kind="ExternalOutput")

    with tile.TileContext(nc) as tc:
        with tc.tile_pool(name="const", bufs=1) as cp:
            xT_sb = cp.tile([P, DMT * M], bf, tag="xT")
            wqkpq_sb = cp.tile([P, DMT * P], bf, tag="wqkpq")
            wqkpk_sb = cp.tile([P, DMT * P], bf, tag="wqkpk")
            wqkr_sb = cp.tile([P, DMT * 6 * P], bf, tag="wqkr")
            wv_sb = cp.tile([P, DMT * FV], bf, tag="wv")
            wout_sb = cp.tile([P, HT * DM], bf, tag="wout")
            vones_sb = cp.tile([P, MT * HL], bf, tag="vones")
            ident = cp.tile([P, P], bf, tag="ident")
            vplus = cp.tile([P, MT * VROW], bf, tag="vplus")
            qk_all = cp.tile([P, HL * M], bf, tag="qkall")
            attT = cp.tile([P, HT * IEFF], bf, tag="attT")
            part01 = cp.tile([P, IT * DM], bf, tag="part01")

            # Single queue, consumption order; xT per-dmt so phase-1 QK
            # pipelines with the serial DMA stream.
            nc.sync.dma_start(out=wqkpq_sb[:, :], in_=wqkpq[:, :])
            nc.sync.dma_start(out=xT_sb[:, ts(0, M)], in_=xT[:, ts(0, M)])
            nc.sync.dma_start(out=wqkpk_sb[:, :], in_=wqkpk[:, :])
            for dmt in range(1, DMT):
                nc.sync.dma_start(out=xT_sb[:, ts(dmt, M)], in_=xT[:, ts(dmt, M)])
            nc.sync.dma_start(out=wv_sb[:, :], in_=wv[:, :])
            nc.sync.dma_start(out=vones_sb[:, :], in_=vones[:, :])
            nc.sync.dma_start(out=wqkr_sb[:, :], in_=wqkr[:, :])
            nc.sync.dma_start(out=wout_sb[:, :], in_=wout[:, :])
            make_identity(nc, ident)

            def wqk_slice(ft, dmt):
                """Stationary [128, 128] weight tile for feature-tile ft."""
                if ft == 0:
                    return wqkpq_sb[:, ts(dmt, P)]
                if ft == NP:
                    return wqkpk_sb[:, ts(dmt, P)]
                ridx = ft - 1 if ft < NP else ft - 2
                return wqkr_sb[:, dmt * 6 * P + ridx * P : dmt * 6 * P + (ridx + 1) * P]

            vp4 = vplus.rearrange("p (t g c) -> p t g c", t=MT, g=HL, c=VW)
            TSLOTS = 30
            tstore = cp.tile([P, TSLOTS * IEFF], bf, tag="tstore")
            tslot_ctr = [0]

            with (
                tc.tile_pool(name="psqk", bufs=2, space="PSUM") as pqk,
                tc.tile_pool(name="pss", bufs=2, space="PSUM") as pss,
                tc.tile_pool(name="psa", bufs=2, space="PSUM") as psa,
                tc.tile_pool(name="appool", bufs=2) as app,
                tc.tile_pool(name="spool", bufs=6) as sp,
            ):

                class Filler:
                    """FIFO of single-matmul-sized PE work units: pair-1..3
                    QK projection, then out-projection ht0/ht1 chunks."""

                    def __init__(self):
                        self.items = [
                            ("qk", ft, g0, gw, dmt)
                            for pair in range(1, NP)
                            for ft in (pair, NP + pair)
                            for (g0, gw) in (groups_q if ft < NP else groups)
                            for dmt in range(DMT)
                        ] + [
                            ("opA", it, ch, s)
                            for it in range(MT)
                            for ch in range(2)
                            for s in range(2)
                        ]
                        self.pos = 0
                        self.ps = None
                        per_pair = (len(groups_q) + len(groups)) * DMT
                        self.qk_end = {
                            pair: pair * per_pair for pair in range(1, NP)
                        }

                    def emit(self, n):
                        for _ in range(n):
                            if self.pos >= len(self.items):
                                return
                            item = self.items[self.pos]
                            if item[0] == "qk":
                                _, ft, g0, gw, dmt = item
                                if dmt == 0:
                                    self.ps = pqk.tile(
                                        [P, 512], f32, tag="qk", name="ps_qk"
                                    )
                                nc.tensor.matmul(
                                    self.ps[:, 0:gw],
                                    wqk_slice(ft, dmt),
                                    xT_sb[:, dmt * M + g0 : dmt * M + g0 + gw],
                                    start=(dmt == 0),
                                    stop=(dmt == DMT - 1),
                                )
                                if dmt == DMT - 1:
                                    nc.vector.tensor_copy(
                                        qk_all[:, ft * M + g0 : ft * M + g0 + gw],
                                        self.ps[:, 0:gw],
                                    )
                                    self.ps = None
                            else:
                                _, it, ch, s = item
                                if s == 0:
                                    self.ps = pqk.tile(
                                        [P, 512], f32, tag="qk", name="ps_oa"
                                    )
                                nc.tensor.matmul(
                                    self.ps[:, :],
                                    attT[:, s * M + it * P : s * M + (it + 1) * P],
                                    wout_sb[
                                        :, s * DM + ch * 512 : s * DM + (ch + 1) * 512
                                    ],
                                    start=(s == 0),
                                    stop=(s == 1),
                                )
                                if s == 1:
                                    eng = nc.vector
                                    eng.tensor_copy(
                                        part01[
                                            :,
                                            it * DM + ch * 512 : it * DM + (ch + 1) * 512,
                                        ],
                                        self.ps[:, :],
                                    )
                                    self.ps = None
                            self.pos += 1

                    def drain_qk_pair(self, pair):
                        if pair in self.qk_end:
                            while self.pos < self.qk_end[pair]:
                                self.emit(1)

                # Pre-zero the t-ring columns an exp may leave unwritten
                # (beyond the effective i-width) so PV always reads
                # initialized data; runs on the otherwise-idle Pool engine
                # during the DMA-bound startup.
                if WEFFJ < IEFF:
                    tst3 = tstore.rearrange("p (k w) -> p k w", k=TSLOTS, w=IEFF)
                    nc.gpsimd.memset(tst3[:, :, WEFFJ:IEFF], 1.0)

                # PE p-state warm-up on the psa ring (idle until the head
                # loop): the Tensor engine ramps to full clock over ~3us
                # of continuous work, so burn the DMA-bound startup on
                # dummy transposes instead of ramping during phase 1.
                warm = psa.tile([P, P], bf, tag="att", name="warm")
                for _ in range(3):
                    nc.tensor.transpose(warm[:, :], ident, ident)

                # ---- phase 1: QK projection for pair 0 ----
                # dmt-outer over 4 concurrent psum groups (pss slots are
                # idle here) so the PE consumes each xT tile as the serial
                # DMA stream delivers it; remaining groups in a quick
                # second pass once everything is resident.
                p1a = [(0, groups_q[0]), (0, groups_q[1]), (NP, groups[0]),
                       (NP, groups[1])]
                p1b = [(0, g) for g in groups_q[2:]] + [
                    (NP, g) for g in groups[2:]
                ]
                ps_map = {}
                for i, (ft, (g0, gw)) in enumerate(p1a):
                    pool = pqk if i < 2 else pss
                    shape = [P, 512] if i < 2 else [P, IEFF]
                    tag = "qk" if i < 2 else "s"
                    ps_map[(ft, g0)] = pool.tile(shape, f32, tag=tag, name="ps_p1")
                for dmt in range(DMT):
                    for (ft, (g0, gw)) in p1a:
                        nc.tensor.matmul(
                            ps_map[(ft, g0)][:, 0:gw],
                            wqk_slice(ft, dmt),
                            xT_sb[:, dmt * M + g0 : dmt * M + g0 + gw],
                            start=(dmt == 0),
                            stop=(dmt == DMT - 1),
                        )
                for (ft, (g0, gw)) in p1a:
                    nc.vector.tensor_copy(
                        qk_all[:, ft * M + g0 : ft * M + g0 + gw],
                        ps_map[(ft, g0)][:, 0:gw],
                    )
                for (ft, (g0, gw)) in p1b:
                    ps = pqk.tile([P, 512], f32, tag="qk", name="ps_qk")
                    for dmt in range(DMT):
                        nc.tensor.matmul(
                            ps[:, 0:gw],
                            wqk_slice(ft, dmt),
                            xT_sb[:, dmt * M + g0 : dmt * M + g0 + gw],
                            start=(dmt == 0),
                            stop=(dmt == DMT - 1),
                        )
                    nc.vector.tensor_copy(
                        qk_all[:, ft * M + g0 : ft * M + g0 + gw], ps[:, 0:gw]
                    )

                def score_exp(h, jt):
                    p0 = (h % 2) * 64
                    pair = h // 2
                    ps_s = pss.tile([P, IEFF], f32, tag="s", name="ps_s")
                    kT = qk_all[
                        p0 : p0 + 64,
                        (NP + pair) * M + jt * P : (NP + pair) * M + (jt + 1) * P,
                    ]
                    for (off, w) in jchunks:
                        c0 = pair * M + off
                        nc.tensor.matmul(
                            ps_s[:, off : off + w],
                            kT,
                            qk_all[p0 : p0 + 64, c0 : c0 + w],
                            start=True,
                            stop=True,
                        )
                    slot = tslot_ctr[0] % TSLOTS
                    tslot_ctr[0] += 1
                    t = tstore[:, slot * IEFF : (slot + 1) * IEFF]
                    nc.scalar.activation(t[:, 0:WEFFJ], ps_s[:, 0:WEFFJ], EXP)
                    return t

                def pv(h, base, jt, t, pa, ntiles):
                    for it in range(ntiles):
                        nc.tensor.matmul(
                            pa[:, it * VW : (it + 1) * VW],
                            t[:, (base + it) * P : (base + it + 1) * P],
                            vplus[:, jt * VROW + h * VW : jt * VROW + (h + 1) * VW],
                            start=(jt == 0 and it == 0),
                            stop=(jt == MT - 1 and it == ntiles - 1),
                        )

                # ---- phase 2: head-0 scores/exp + V projection ----
                tmap = {}
                for jt in range(MT):
                    tmap[(0, jt)] = score_exp(0, jt)
                    ps_v = pqk.tile([P, 512], f32, tag="qk", name="ps_v")
                    for dmt in range(DMT):
                        nc.tensor.matmul(
                            ps_v[:, :],
                            xT_sb[:, dmt * M + jt * P : dmt * M + (jt + 1) * P],
                            wv_sb[:, ts(dmt, FV)],
                            start=(dmt == 0),
                            stop=(dmt == DMT - 1),
                        )
                    nc.vector.tensor_copy(
                        vp4[:, jt, :, 0:DH],
                        ps_v.rearrange("p (g c) -> p g c", g=HL, c=DH),
                    )
                    nc.gpsimd.tensor_copy(
                        vp4[:, jt, :, DH], vones_sb[:, jt * HL : (jt + 1) * HL]
                    )

                def transpose_half(pair, ap_tile, base, ntiles):
                    """Per-half transposes, batched 2 per psum tile, DVE
                    evictions (bf16 2x mode)."""
                    for it0 in range(base, base + ntiles, 2):
                        nb = min(2, base + ntiles - it0)
                        ps_tr = pqk.tile([P, 2 * P], bf, tag="qk", name="ps_tr")
                        for k in range(nb):
                            nc.tensor.transpose(
                                ps_tr[:, k * P : (k + 1) * P],
                                ap_tile[:, ts(it0 + k, P)],
                                ident,
                            )
                        nc.vector.tensor_copy(
                            attT[:, pair * M + it0 * P : pair * M + (it0 + nb) * P],
                            ps_tr[:, 0 : nb * P],
                        )

                def out_proj_b(its):
                    """Final output projection (ht3 + SBUF partial of
                    ht0..ht2) for the given i-tiles; psum alternates
                    pqk/pss rings, output DMAs alternate queues."""
                    for it in its:
                        o_sb = sp.tile([P, DM], bf, tag="ob", name="o_sb")
                        for ch in range(2):
                            if ch == 0:
                                ps_o = pqk.tile([P, 512], f32, tag="qk", name="ps_o")
                            else:
                                ps_o = pss.tile([P, IEFF], f32, tag="s", name="ps_o")
                            for ht in (2, 3):
                                nc.tensor.matmul(
                                    ps_o[:, 0:512],
                                    attT[:, ht * M + it * P : ht * M + (it + 1) * P],
                                    wout_sb[
                                        :,
                                        ht * DM + ch * 512 : ht * DM + (ch + 1) * 512,
                                    ],
                                    start=(ht == 2),
                                    stop=(ht == 3 and ch == 1),
                                )
                            if ch == 0:
                                # identity matmul folds the ht0/ht1 SBUF
                                # partial into the psum accumulation on PE,
                                # then ACT (idle here) evicts with a copy
                                nc.tensor.matmul(
                                    ps_o[:, 0:512],
                                    ident,
                                    part01[:, it * DM : it * DM + 512],
                                    start=False,
                                    stop=True,
                                )
                                COPY = mybir.ActivationFunctionType.Copy
                                nc.scalar.activation(
                                    o_sb[:, 0:512], ps_o[:, 0:512], COPY
                                )
                            else:
                                nc.vector.tensor_add(
                                    o_sb[:, 512:DM],
                                    ps_o[:, 0:512],
                                    part01[:, it * DM + 512 : (it + 1) * DM],
                                )
                        nc.sync.dma_start(out=out[ts(it, P), :], in_=o_sb[:, :])

                nc.sync.dma_start(out=vpx[:, :], in_=vplus[:, :])

                # ---- attention blocks with one-block score lookahead ----
                blocks = [(h, half) for h in range(HL) for half in (0, 1)]
                # filler budget per block: pair p+1's QK must complete
                # before block 2*(p+1) emits its lookahead scores; opA
                # (out-projection ht0/ht1) fills b10..b13.
                budgets = [20, 20] + [10] * 8 + [9, 7, 6, 6] + [0, 0]
                filler = Filler()
                ap_tile = None
                for bi, (h, half) in enumerate(blocks):
                    pair = h // 2
                    p0 = (h % 2) * 64
                    if half == 0:
                        if h % 2 == 0:
                            ap_tile = app.tile([P, MT * P], bf, tag="ap", name="ap")
                    ntiles = IHA if half == 0 else IHB
                    base = 0 if half == 0 else IHA
                    pa = psa.tile([P, IHA * VW], f32, tag="att", name="pa")
                    pa3 = pa.rearrange("p (t c) -> p t c", t=IHA, c=VW)
                    # the lookahead scores one head out may start a new
                    # head pair: its QK projection must be complete now.
                    if half == 0 and h + 1 < HL:
                        filler.drain_qk_pair((h + 1) // 2)
                    if bi == 10:
                        # k fully projected by now: export for the host
                        nc.sync.dma_start(
                            out=kx[:, :], in_=qk_all[:, NP * M : 2 * NP * M]
                        )
                    budget = budgets[bi]
                    sched, acc = [], 0
                    for jt in range(MT):
                        acc += budget
                        sched.append(acc // MT - (acc - budget) // MT)
                    # lookahead emission for head h+1, halves interleaved
                    # so each block gets a balanced mix of wide (halfA)
                    # and narrow (halfB) exps on ACT
                    seq = [(hf2, jt2) for jt2 in range(MT) for hf2 in (0, 1)]
                    soff = 0 if half == 0 else MT
                    for jt in range(MT):
                        pv(h, half, jt, tmap.pop((h, half, jt)), pa, ntiles)
                        if h + 1 < HL:
                            hf2, jt2 = seq[soff + jt]
                            tmap[(h + 1, hf2, jt2)] = score_exp(h + 1, hf2, jt2)
                        filler.emit(sched[jt])
                    r = sp.tile([P, IHA], f32, tag="r", name="r_sb")
                    nc.vector.reciprocal(r[:, 0:ntiles], pa3[:, 0:ntiles, DH])
                    for it in range(ntiles):
                        eng = nc.vector
                        eng.tensor_scalar_mul(
                            ap_tile[
                                :, (base + it) * P + p0 : (base + it) * P + p0 + DH
                            ],
                            pa3[:, it, 0:DH],
                            r[:, it : it + 1],
                        )
                    if h % 2 == 1:
                        filler.drain_qk_pair(pair + 1)
                        transpose_half(pair, ap_tile, base, ntiles)
                        if pair == NP - 1:
                            filler.emit(len(filler.items))  # safety
                            out_proj_b(range(base, base + ntiles))

    nc.compile()
    return nc


def _compact(mask):
    """Live indices per batch and the padded capacity M (multiple of 128)."""
    live = [np.flatnonzero(np.asarray(mask[b]) != 0) for b in range(B)]
    mmax = max(1, max(len(l) for l in live))
    M = -(-mmax // P) * P
    return live, M


def _shard_inputs(x, w_qkv, w_out, b_out, mask):
    """Build the per-core input maps (host-side compaction + layout prep)."""
    bf = ml_dtypes.bfloat16
    x = np.asarray(x, dtype=np.float32)
    w_qkv = np.asarray(w_qkv, dtype=np.float32)
    w_out = np.asarray(w_out, dtype=np.float32)
    mask = np.asarray(mask)

    live, M = _compact(mask)
    MT = M // P

    def sbimg(a):
        """[DMT*P, C] -> SBUF image [P, DMT*C]."""
        r, ccols = a.shape
        d = r // P
        return np.ascontiguousarray(
            a.reshape(d, P, ccols).transpose(1, 0, 2).reshape(P, d * ccols)
        )

    # w_qkv columns: head h occupies cols [h*192, (h+1)*192) as q|k|v of 64.
    w3 = w_qkv.reshape(DM, H, 3, DH)
    in_maps = []
    for c in range(NCORES):
        b, hg = c // HG, c % HG
        idx = live[b]
        ml = len(idx)
        wq = w3[:, hg * HL : (hg + 1) * HL, 0, :].reshape(DM, FV) * SCALE
        wk = w3[:, hg * HL : (hg + 1) * HL, 1, :].reshape(DM, FV)
        # feature-tile ft of the q|k concat: q tiles 0..NP-1 hold head
        # pairs, k tiles NP..2NP-1 likewise.  Priority tensor = ft0|ft4
        # per dmt; rest tensor = ft 1,2,3,5,6,7 per dmt.
        fts = np.concatenate([wq, wk], axis=1).reshape(DM, 2 * NP, P)
        wqkpq_c = sbimg(np.ascontiguousarray(fts[:, 0])).astype(bf)
        wqkpk_c = sbimg(np.ascontiguousarray(fts[:, NP])).astype(bf)
        rest_idx = [1, 2, 3, NP + 1, NP + 2, NP + 3]
        wqkr_c = sbimg(
            np.concatenate([fts[:, i] for i in rest_idx], axis=1)
        ).astype(bf)
        wv_c = sbimg(
            np.ascontiguousarray(
                w3[:, hg * HL : (hg + 1) * HL, 2, :].reshape(DM, FV)
            )
        ).astype(bf)
        wout_c = sbimg(
            np.ascontiguousarray(w_out[hg * FV : (hg + 1) * FV, :])
        ).astype(bf)
        xT_c = np.zeros((DM, M), dtype=np.float32)
        xT_c[:, :ml] = x[b][idx].T
        xT_c = sbimg(xT_c).astype(bf)
        ones = np.zeros((M,), np.float32)
        ones[:ml] = 1.0
        vones_c = np.ascontiguousarray(
            np.repeat(ones.reshape(MT, P).T[:, :, None], HL, axis=2).reshape(
                P, MT * HL
            )
        ).astype(bf)
        in_maps.append(
            {
                "xT": xT_c,
                "wqkpq": wqkpq_c,
                "wqkpk": wqkpk_c,
                "wqkr": wqkr_c,
                "wv": wv_c,
                "wout": wout_c,
                "vones": vones_c,
            }
        )
    return in_maps, live, M


def kernel(x, w_qkv, w_out, b_out, mask):
    from concourse.bass_utils import run_bass_kernel_spmd

    x = np.asarray(x, dtype=np.float32)
    w_qkv = np.asarray(w_qkv, dtype=np.float32)
    w_out_f = np.asarray(w_out, dtype=np.float32)
    b_out = np.asarray(b_out, dtype=np.float32)
    mask = np.asarray(mask)

    in_maps, live, M = _shard_inputs(x, w_qkv, w_out_f, b_out, mask)
    meff = -(-max(1, max(len(l) for l in live)) // 32) * 32
    key = (M, meff)
    if key not in _CACHE:
        _CACHE[key] = _build_program(M, meff)
    nc = _CACHE[key]

    res = run_bass_kernel_spmd(nc, in_maps, list(range(NCORES))).results

    MT = M // P
    IT = min(MT, 8)
    IEFF = IT * P
    w3d = w_qkv.astype(np.float64).reshape(DM, H, 3, DH)
    wv_full = w3d[:, :, 2, :].reshape(DM, H * DH)
    w3 = w_qkv.reshape(DM, H, 3, DH)
    out = np.empty((B, N, DM), np.float32)
    for b in range(B):
        idx = live[b]
        ml = len(idx)
        mldev = min(ml, IEFF)
        part = res[HG * b]["out"].astype(np.float32)[:mldev] + res[HG * b + 1][
            "out"
        ].astype(np.float32)[:mldev]
        # dead-query rows: uniform softmax over ALL tokens -> mean of v
        # rows through the output projection; exact float64 on the host.
        dead_row = (
            x[b].astype(np.float64).mean(axis=0) @ wv_full @ w_out_f.astype(np.float64)
        ).astype(np.float32)
        out[b] = dead_row[None, :] + b_out[None, :]
        out[b][idx[:mldev]] = part + b_out[None, :]
        if ml > IEFF:
            # overflow live queries (tokens beyond the device i-range):
            # exact attention on the host from the device-exported k / v
            # (bf16, so identical rounding to the device path).
            rows = idx[IEFF:ml]
            xq = x[b][rows]
            acc = np.empty((len(rows), H * DH), np.float32)
            for hg in range(HG):
                c = HG * b + hg
                kxa = res[c]["kx"].astype(np.float32)
                vpa = res[c]["vpx"].astype(np.float32)
                k_loc = (
                    kxa.reshape(P, NP, M).transpose(2, 1, 0).reshape(M, HL * DH)
                )
                v_loc = (
                    vpa.reshape(P, MT, HL, VW)[:, :, :, :DH]
                    .transpose(1, 0, 2, 3)
                    .reshape(M, HL, DH)
                )
                wq = w3[:, hg * HL : (hg + 1) * HL, 0, :].reshape(DM, HL * DH)
                q = (xq @ wq).reshape(-1, HL, DH) * SCALE
                for lh in range(HL):
                    s = q[:, lh] @ k_loc[:ml, lh * DH : (lh + 1) * DH].T
                    t = np.exp(s)
                    o = (t @ v_loc[:ml, lh]) / t.sum(axis=1)[:, None]
                    acc[:, (hg * HL + lh) * DH : (hg * HL + lh + 1) * DH] = o
            out[b][rows] = acc @ w_out_f + b_out[None, :]
    return out


# revision 81
# speedup vs baseline: 3.3541x; 1.0007x over previous
"""Multi-head attention (B=4, N=2048, DM=1024, H=16, DH=64) on 8 trn2 cores.

Sharding: core c -> (batch b = c//2, head-group hg = c%2 of 8 heads).

Live-token compaction: the pair mask m_i*m_j means masked keys contribute
exactly zero to every live query's softmax (exp(-1e6) == 0 in f32), and
masked queries get the uniform average of ALL values.  So:
  - host compacts each batch to its ~N/2 live tokens (padded to M, a
    multiple of 128), and the device runs plain UNMASKED attention on the
    compacted tokens: scores/exp/PV shrink ~(M/N)^2 ~ 3.2x, projections
    ~2x.  Padded tokens have x=0 and a zeroed "ones" column in v-plus, so
    they add exactly 0 to both the numerator and the softmax denominator.
  - dead-query rows (identical for every dead i within a batch: the
    uniform-softmax average of v over all 2048 tokens through the output
    projection) are computed exactly on the host in float64.

Device schedule (PE-bound; ACT exp is the secondary engine):
  - feature-major x^T so QK projection emits q^T/k^T [64, M] per head;
    SCALE pre-baked into w_q; v token-major with a live-flag column so PV
    accumulates the softmax denominator for free.
  - i-dim in two halves (ceil/floor of M/128 tiles); exp at [128, half]
    width on ACT.
  - scores+exp for block i+2 are emitted during block i (one-block
    lookahead) so PV never waits on ACT.
  - a filler FIFO streams pair-1..3 QK projection matmuls and the first
    half (ht0/ht1) of the output projection into the PE slack of the
    ACT-bound attention blocks; the final output projection only
    contracts ht2/ht3 and adds the SBUF partial back in.
  - inputs are stored in SBUF-image layout and loaded with a few large
    DMAs in consumption order (HWDGE descriptor generation is a serial
    ~630ns/DMA resource; DMA engines share one serial ~360B/ns pipe).
"""

import sys

sys.path.insert(0, "/opt/trn_rl_repo")

import numpy as np
import ml_dtypes

B, N, DM, H, DH = 4, 2048, 1024, 16, 64
SCALE = DH**-0.5
NCORES = 8
HG = 2  # head groups (tensor-parallel factor)
HL = H // HG  # 8 heads per core
NP = HL // 2  # 4 head pairs
FQK = HL * 2 * DH  # 1024 qk features per core
FV = HL * DH  # 512 v features per core
P = 128
DMT = DM // P  # 8 dm tiles
VW = DH + 1  # 65: v columns + live-flag column
HT = FV // P  # 4 head-dim tiles for the projection

_CACHE = {}


def _even_groups(total, maxw):
    """Split `total` into contiguous (offset, width) groups of width<=maxw."""
    n = -(-total // maxw)
    base, rem = divmod(total, n)
    out, off = [], 0
    for i in range(n):
        w = base + (1 if i < rem else 0)
        out.append((off, w))
        off += w
    return out


def _build_program(M, MEFF):
    import concourse.mybir as mybir
    import concourse.tile as tile
    from concourse import bacc
    from concourse.bass import ts
    from concourse.masks import make_identity

    bf = mybir.dt.bfloat16
    f32 = mybir.dt.float32
    EXP = mybir.ActivationFunctionType.Exp

    MT = M // P  # token (key) tiles
    VROW = HL * VW
    groups = _even_groups(M, 512)  # token chunks for projections

    def _bank_chunks(total):
        """512-aligned (offset, width) chunks: matmul outputs must not
        cross a PSUM bank (512 f32) boundary."""
        out, off = [], 0
        while off < total:
            out.append((off, min(512, total - off)))
            off += 512
        return out

    # Device computes queries for the first IEFF=min(M,1024) tokens only
    # (overflow live queries are handled on the host from exported k/v).
    # This makes the joint scores psum tile [P, IEFF] exactly 2 banks, so
    # ONE exp per (head, j-tile) instead of two.
    IT = min(MT, 8)  # device i-tiles
    IEFF = IT * P
    IHA = (IT + 1) // 2
    IHB = IT - IHA
    WEFFJ = max(32, min(IEFF, MEFF))  # trimmed exp width
    jchunks = _bank_chunks(WEFFJ)
    # q-side projection columns beyond WEFFJ are never read by scores
    groups_q = _bank_chunks(min(M, max(512, WEFFJ)))

    nc = bacc.Bacc(
        "TRN2", target_bir_lowering=False, debug=False, num_devices=NCORES
    )
    # All inputs are stored in SBUF-image layout [128, cols] (host
    # pre-swizzles) so each tensor loads with one (or a few) large DMAs:
    # the HWDGE descriptor generator is a serial ~630ns/DMA resource and
    # the DMA engines share one serial 360B/ns pipe, so count and order
    # are what matter.  wqk is split into the pair-0 columns (needed
    # first) and the rest.
    xT = nc.dram_tensor("xT", [P, DMT * M], bf, kind="ExternalInput")
    wqkpq = nc.dram_tensor("wqkpq", [P, DMT * P], bf, kind="ExternalInput")
    wqkpk = nc.dram_tensor("wqkpk", [P, DMT * P], bf, kind="ExternalInput")
    wqkr = nc.dram_tensor("wqkr", [P, DMT * 6 * P], bf, kind="ExternalInput")
    wv = nc.dram_tensor("wv", [P, DMT * FV], bf, kind="ExternalInput")
    wout = nc.dram_tensor("wout", [P, HT * DM], bf, kind="ExternalInput")
    vones = nc.dram_tensor("vones", [P, MT * HL], bf, kind="ExternalInput")
    out = nc.dram_tensor("out", [IEFF, DM], bf, kind="ExternalOutput")
    kx = nc.dram_tensor("kx", [P, NP * M], bf, kind="ExternalOutput")
    vpx = nc.dram_tensor("vpx", [P, MT * VROW], bf, kind="ExternalOutput")

    with tile.TileContext(nc) as tc:
        with tc.tile_pool(name="const", bufs=1) as cp:
            xT_sb = cp.tile([P, DMT * M], bf, tag="xT")
            wqkpq_sb = cp.tile([P, DMT * P], bf, tag="wqkpq")
            wqkpk_sb = cp.tile([P, DMT * P], bf, tag="wqkpk")
            wqkr_sb = cp.tile([P, DMT * 6 * P], bf, tag="wqkr")
            wv_sb = cp.tile([P, DMT * FV], bf, tag="wv")
            wout_sb = cp.tile([P, HT * DM], bf, tag="wout")
            vones_sb = cp.tile([P, MT * HL], bf, tag="vones")
            ident = cp.tile([P, P], bf, tag="ident")
            vplus = cp.tile([P, MT * VROW], bf, tag="vplus")
            qk_all = cp.tile([P, HL * M], bf, tag="qkall")
            attT = cp.tile([P, HT * IEFF], bf, tag="attT")
            part01 = cp.tile([P, IT * DM], bf, tag="part01")

            # Single queue, consumption order; xT per-dmt so phase-1 QK
            # pipelines with the serial DMA stream.
            nc.sync.dma_start(out=wqkpq_sb[:, :], in_=wqkpq[:, :])
            nc.sync.dma_start(out=xT_sb[:, ts(0, M)], in_=xT[:, ts(0, M)])
            nc.sync.dma_start(out=wqkpk_sb[:, :], in_=wqkpk[:, :])
            for dmt in range(1, DMT):
                nc.sync.dma_start(out=xT_sb[:, ts(dmt, M)], in_=xT[:, ts(dmt, M)])
            nc.sync.dma_start(out=wv_sb[:, :], in_=wv[:, :])
            nc.sync.dma_start(out=vones_sb[:, :], in_=vones[:, :])
            nc.sync.dma_start(out=wqkr_sb[:, :], in_=wqkr[:, :])
            nc.sync.dma_start(out=wout_sb[:, :], in_=wout[:, :])
            make_identity(nc, ident)

            def wqk_slice(ft, dmt):
                """Stationary [128, 128] weight tile for feature-tile ft."""
                if ft == 0:
                    return wqkpq_sb[:, ts(dmt, P)]
                if ft == NP:
                    return wqkpk_sb[:, ts(dmt, P)]
                ridx = ft - 1 if ft < NP else ft - 2
                return wqkr_sb[:, dmt * 6 * P + ridx * P : dmt * 6 * P + (ridx + 1) * P]

            vp4 = vplus.rearrange("p (t g c) -> p t g c", t=MT, g=HL, c=VW)
            TSLOTS = 34
            tstore = cp.tile([P, TSLOTS * IEFF], bf, tag="tstore")
            tslot_ctr = [0]

            with (
                tc.tile_pool(name="psqk", bufs=2, space="PSUM") as pqk,
                tc.tile_pool(name="pss", bufs=2, space="PSUM") as pss,
                tc.tile_pool(name="psa", bufs=2, space="PSUM") as psa,
                tc.tile_pool(name="appool", bufs=2) as app,
                tc.tile_pool(name="spool", bufs=6) as sp,
            ):

                class Filler:
                    """FIFO of single-matmul-sized PE work units: pair-1..3
                    QK projection, then out-projection ht0/ht1 chunks."""

                    def __init__(self):
                        self.items = [
                            ("qk", ft, g0, gw, dmt)
                            for pair in range(1, NP)
                            for ft in (pair, NP + pair)
                            for (g0, gw) in (groups_q if ft < NP else groups)
                            for dmt in range(DMT)
                        ] + [
                            ("opA", it, ch, s)
                            for it in range(MT)
                            for ch in range(2)
                            for s in range(2)
                        ]
                        self.pos = 0
                        self.ps = None
                        per_pair = (len(groups_q) + len(groups)) * DMT
                        self.qk_end = {
                            pair: pair * per_pair for pair in range(1, NP)
                        }

                    def emit(self, n):
                        for _ in range(n):
                            if self.pos >= len(self.items):
                                return
                            item = self.items[self.pos]
                            if item[0] == "qk":
                                _, ft, g0, gw, dmt = item
                                if dmt == 0:
                                    self.ps = pqk.tile(
                                        [P, 512], f32, tag="qk", name="ps_qk"
                                    )
                                nc.tensor.matmul(
                                    self.ps[:, 0:gw],
                                    wqk_slice(ft, dmt),
                                    xT_sb[:, dmt * M + g0 : dmt * M + g0 + gw],
                                    start=(dmt == 0),
                                    stop=(dmt == DMT - 1),
                                )
                                if dmt == DMT - 1:
                                    nc.vector.tensor_copy(
                                        qk_all[:, ft * M + g0 : ft * M + g0 + gw],
                                        self.ps[:, 0:gw],
                                    )
                                    self.ps = None
                            else:
                                _, it, ch, s = item
                                if s == 0:
                                    self.ps = pqk.tile(
                                        [P, 512], f32, tag="qk", name="ps_oa"
                                    )
                                nc.tensor.matmul(
                                    self.ps[:, :],
                                    attT[:, s * M + it * P : s * M + (it + 1) * P],
                                    wout_sb[
                                        :, s * DM + ch * 512 : s * DM + (ch + 1) * 512
                                    ],
                                    start=(s == 0),
                                    stop=(s == 1),
                                )
                                if s == 1:
                                    eng = nc.vector
                                    eng.tensor_copy(
                                        part01[
                                            :,
                                            it * DM + ch * 512 : it * DM + (ch + 1) * 512,
                                        ],
                                        self.ps[:, :],
                                    )
                                    self.ps = None
                            self.pos += 1

                    def drain_qk_pair(self, pair):
                        if pair in self.qk_end:
                            while self.pos < self.qk_end[pair]:
                                self.emit(1)

                # Pre-zero the t-ring columns an exp may leave unwritten
                # (beyond the effective i-width) so PV always reads
                # initialized data; runs on the otherwise-idle Pool engine
                # during the DMA-bound startup.
                if WEFFJ < IEFF:
                    tst3 = tstore.rearrange("p (k w) -> p k w", k=TSLOTS, w=IEFF)
                    nc.gpsimd.memset(tst3[:, :, WEFFJ:IEFF], 1.0)

                # PE p-state warm-up on the psa ring (idle until the head
                # loop): the Tensor engine ramps to full clock over ~3us
                # of continuous work, so burn the DMA-bound startup on
                # dummy transposes instead of ramping during phase 1.
                warm = psa.tile([P, P], bf, tag="att", name="warm")
                for _ in range(3):
                    nc.tensor.transpose(warm[:, :], ident, ident)

                # ---- phase 1: QK projection for pair 0 ----
                # dmt-outer over 4 concurrent psum groups (pss slots are
                # idle here) so the PE consumes each xT tile as the serial
                # DMA stream delivers it; remaining groups in a quick
                # second pass once everything is resident.
                p1a = [(0, groups_q[0]), (0, groups_q[1]), (NP, groups[0]),
                       (NP, groups[1])]
                p1b = [(0, g) for g in groups_q[2:]] + [
                    (NP, g) for g in groups[2:]
                ]
                ps_map = {}
                for i, (ft, (g0, gw)) in enumerate(p1a):
                    pool = pqk if i < 2 else pss
                    shape = [P, 512] if i < 2 else [P, IEFF]
                    tag = "qk" if i < 2 else "s"
                    ps_map[(ft, g0)] = pool.tile(shape, f32, tag=tag, name="ps_p1")
                for dmt in range(DMT):
                    for (ft, (g0, gw)) in p1a:
                        nc.tensor.matmul(
                            ps_map[(ft, g0)][:, 0:gw],
                            wqk_slice(ft, dmt),
                            xT_sb[:, dmt * M + g0 : dmt * M + g0 + gw],
                            start=(dmt == 0),
                            stop=(dmt == DMT - 1),
                        )
                for (ft, (g0, gw)) in p1a:
                    nc.vector.tensor_copy(
                        qk_all[:, ft * M + g0 : ft * M + g0 + gw],
                        ps_map[(ft, g0)][:, 0:gw],
                    )
                for (ft, (g0, gw)) in p1b:
                    ps = pqk.tile([P, 512], f32, tag="qk", name="ps_qk")
                    for dmt in range(DMT):
                        nc.tensor.matmul(
                            ps[:, 0:gw],
                            wqk_slice(ft, dmt),
                            xT_sb[:, dmt * M + g0 : dmt * M + g0 + gw],
                            start=(dmt == 0),
                            stop=(dmt == DMT - 1),
                        )
                    nc.vector.tensor_copy(
                        qk_all[:, ft * M + g0 : ft * M + g0 + gw], ps[:, 0:gw]
                    )

                def score_exp(h, jt):
                    p0 = (h % 2) * 64
                    pair = h // 2
                    ps_s = pss.tile([P, IEFF], f32, tag="s", name="ps_s")
                    kT = qk_all[
                        p0 : p0 + 64,
                        (NP + pair) * M + jt * P : (NP + pair) * M + (jt + 1) * P,
                    ]
                    for (off, w) in jchunks:
                        c0 = pair * M + off
                        nc.tensor.matmul(
                            ps_s[:, off : off + w],
                            kT,
                            qk_all[p0 : p0 + 64, c0 : c0 + w],
                            start=True,
                            stop=True,
                        )
                    slot = tslot_ctr[0] % TSLOTS
                    tslot_ctr[0] += 1
                    t = tstore[:, slot * IEFF : (slot + 1) * IEFF]
                    nc.scalar.activation(t[:, 0:WEFFJ], ps_s[:, 0:WEFFJ], EXP)
                    return t

                def pv(h, base, jt, t, pa, ntiles):
                    for it in range(ntiles):
                        nc.tensor.matmul(
                            pa[:, it * VW : (it + 1) * VW],
                            t[:, (base + it) * P : (base + it + 1) * P],
                            vplus[:, jt * VROW + h * VW : jt * VROW + (h + 1) * VW],
                            start=(jt == 0 and it == 0),
                            stop=(jt == MT - 1 and it == ntiles - 1),
                        )

                # ---- phase 2: head-0 scores/exp + V projection ----
                tmap = {}
                for jt in range(MT):
                    tmap[(0, jt)] = score_exp(0, jt)
                    ps_v = pqk.tile([P, 512], f32, tag="qk", name="ps_v")
                    for dmt in range(DMT):
                        nc.tensor.matmul(
                            ps_v[:, :],
                            xT_sb[:, dmt * M + jt * P : dmt * M + (jt + 1) * P],
                            wv_sb[:, ts(dmt, FV)],
                            start=(dmt == 0),
                            stop=(dmt == DMT - 1),
                        )
                    nc.vector.tensor_copy(
                        vp4[:, jt, :, 0:DH],
                        ps_v.rearrange("p (g c) -> p g c", g=HL, c=DH),
                    )
                    nc.gpsimd.tensor_copy(
                        vp4[:, jt, :, DH], vones_sb[:, jt * HL : (jt + 1) * HL]
                    )

                def transpose_half(pair, ap_tile, base, ntiles):
                    """Per-half transposes, batched 2 per psum tile, DVE
                    evictions (bf16 2x mode)."""
                    for it0 in range(base, base + ntiles, 2):
                        nb = min(2, base + ntiles - it0)
                        ps_tr = pqk.tile([P, 2 * P], bf, tag="qk", name="ps_tr")
                        for k in range(nb):
                            nc.tensor.transpose(
                                ps_tr[:, k * P : (k + 1) * P],
                                ap_tile[:, ts(it0 + k, P)],
                                ident,
                            )
                        nc.vector.tensor_copy(
                            attT[:, pair * M + it0 * P : pair * M + (it0 + nb) * P],
                            ps_tr[:, 0 : nb * P],
                        )

                def out_proj_b(its):
                    """Final output projection (ht3 + SBUF partial of
                    ht0..ht2) for the given i-tiles; psum alternates
                    pqk/pss rings, output DMAs alternate queues."""
                    for it in its:
                        o_sb = sp.tile([P, DM], bf, tag="ob", name="o_sb")
                        for ch in range(2):
                            if ch == 0:
                                ps_o = pqk.tile([P, 512], f32, tag="qk", name="ps_o")
                            else:
                                ps_o = pss.tile([P, IEFF], f32, tag="s", name="ps_o")
                            for ht in (2, 3):
                                nc.tensor.matmul(
                                    ps_o[:, 0:512],
                                    attT[:, ht * M + it * P : ht * M + (it + 1) * P],
                                    wout_sb[
                                        :,
                                        ht * DM + ch * 512 : ht * DM + (ch + 1) * 512,
                                    ],
                                    start=(ht == 2),
                                    stop=(ht == 3 and ch == 1),
                                )
                            if ch == 0:
                                # identity matmul folds the ht0/ht1 SBUF
                                # partial into the psum accumulation on PE,
                                # then ACT (idle here) evicts with a copy
                                nc.tensor.matmul(
                                    ps_o[:, 0:512],
                                    ident,
                                    part01[:, it * DM : it * DM + 512],
                                    start=False,
                                    stop=True,
                                )
                                COPY = mybir.ActivationFunctionType.Copy
                                nc.scalar.activation(
                                    o_sb[:, 0:512], ps_o[:, 0:512], COPY
                                )
                            else:
                                nc.vector.tensor_add(
                                    o_sb[:, 512:DM],
                                    ps_o[:, 0:512],
                                    part01[:, it * DM + 512 : (it + 1) * DM],
                                )
                        nc.sync.dma_start(out=out[ts(it, P), :], in_=o_sb[:, :])

                nc.sync.dma_start(out=vpx[:, :], in_=vplus[:, :])

                # ---- attention blocks with one-block score lookahead ----
                blocks = [(h, half) for h in range(HL) for half in (0, 1)]
                # filler budget per block: pair p+1's QK must complete
                # before block 2*(p+1) emits its lookahead scores; opA
                # (out-projection ht0/ht1) fills b10..b13.
                budgets = [20, 20] + [10] * 8 + [9, 7, 6, 6] + [0, 0]
                filler = Filler()
                ap_tile = None
                for bi, (h, half) in enumerate(blocks):
                    pair = h // 2
                    p0 = (h % 2) * 64
                    if half == 0:
                        if h % 2 == 0:
                            ap_tile = app.tile([P, MT * P], bf, tag="ap", name="ap")
                    ntiles = IHA if half == 0 else IHB
                    base = 0 if half == 0 else IHA
                    pa = psa.tile([P, IHA * VW], f32, tag="att", name="pa")
                    pa3 = pa.rearrange("p (t c) -> p t c", t=IHA, c=VW)
                    # the lookahead scores one head out may start a new
                    # head pair: its QK projection must be complete now.
                    if half == 0 and h + 1 < HL:
                        filler.drain_qk_pair((h + 1) // 2)
                    if bi == 10:
                        # k fully projected by now: export for the host
                        nc.sync.dma_start(
                            out=kx[:, :], in_=qk_all[:, NP * M : 2 * NP * M]
                        )
                    budget = budgets[bi]
                    sched, acc = [], 0
                    for jt in range(MT):
                        acc += budget
                        sched.append(acc // MT - (acc - budget) // MT)
                    # lookahead emission for head h+1, halves interleaved
                    # so each block gets a balanced mix of wide (halfA)
                    # and narrow (halfB) exps on ACT
                    seq = [(hf2, jt2) for jt2 in range(MT) for hf2 in (0, 1)]
                    soff = 0 if half == 0 else MT
                    for jt in range(MT):
                        pv(h, half, jt, tmap.pop((h, half, jt)), pa, ntiles)
                        if h + 1 < HL:
                            hf2, jt2 = seq[soff + jt]
                            tmap[(h + 1, hf2, jt2)] = score_exp(h + 1, hf2, jt2)
                        filler.emit(sched[jt])
                    r = sp.tile([P, IHA], f32, tag="r", name="r_sb")
                    nc.vector.reciprocal(r[:, 0:ntiles], pa3[:, 0:ntiles, DH])
                    for it in range(ntiles):
                        eng = nc.vector
                        eng.tensor_scalar_mul(
                            ap_tile[
                                :, (base + it) * P + p0 : (base + it) * P + p0 + DH
                            ],
                            pa3[:, it, 0:DH],
                            r[:, it : it + 1],
                        )
                    if h % 2 == 1:
                        filler.drain_qk_pair(pair + 1)
                        transpose_half(pair, ap_tile, base, ntiles)
                        if pair == NP - 1:
                            filler.emit(len(filler.items))  # safety
                            out_proj_b(range(base, base + ntiles))

    nc.compile()
    return nc


def _compact(mask):
    """Live indices per batch and the padded capacity M (multiple of 128)."""
    live = [np.flatnonzero(np.asarray(mask[b]) != 0) for b in range(B)]
    mmax = max(1, max(len(l) for l in live))
    M = -(-mmax // P) * P
    return live, M


def _shard_inputs(x, w_qkv, w_out, b_out, mask):
    """Build the per-core input maps (host-side compaction + layout prep)."""
    bf = ml_dtypes.bfloat16
    x = np.asarray(x, dtype=np.float32)
    w_qkv = np.asarray(w_qkv, dtype=np.float32)
    w_out = np.asarray(w_out, dtype=np.float32)
    mask = np.asarray(mask)

    live, M = _compact(mask)
    MT = M // P

    def sbimg(a):
        """[DMT*P, C] -> SBUF image [P, DMT*C]."""
        r, ccols = a.shape
        d = r // P
        return np.ascontiguousarray(
            a.reshape(d, P, ccols).transpose(1, 0, 2).reshape(P, d * ccols)
        )

    # w_qkv columns: head h occupies cols [h*192, (h+1)*192) as q|k|v of 64.
    w3 = w_qkv.reshape(DM, H, 3, DH)
    in_maps = []
    for c in range(NCORES):
        b, hg = c // HG, c % HG
        idx = live[b]
        ml = len(idx)
        wq = w3[:, hg * HL : (hg + 1) * HL, 0, :].reshape(DM, FV) * SCALE
        wk = w3[:, hg * HL : (hg + 1) * HL, 1, :].reshape(DM, FV)
        # feature-tile ft of the q|k concat: q tiles 0..NP-1 hold head
        # pairs, k tiles NP..2NP-1 likewise.  Priority tensor = ft0|ft4
        # per dmt; rest tensor = ft 1,2,3,5,6,7 per dmt.
        fts = np.concatenate([wq, wk], axis=1).reshape(DM, 2 * NP, P)
        wqkpq_c = sbimg(np.ascontiguousarray(fts[:, 0])).astype(bf)
        wqkpk_c = sbimg(np.ascontiguousarray(fts[:, NP])).astype(bf)
        rest_idx = [1, 2, 3, NP + 1, NP + 2, NP + 3]
        wqkr_c = sbimg(
            np.concatenate([fts[:, i] for i in rest_idx], axis=1)
        ).astype(bf)
        wv_c = sbimg(
            np.ascontiguousarray(
                w3[:, hg * HL : (hg + 1) * HL, 2, :].reshape(DM, FV)
            )
        ).astype(bf)
        wout_c = sbimg(
            np.ascontiguousarray(w_out[hg * FV : (hg + 1) * FV, :])
        ).astype(bf)
        xT_c = np.zeros((DM, M), dtype=np.float32)
        xT_c[:, :ml] = x[b][idx].T
        xT_c = sbimg(xT_c).astype(bf)
        ones = np.zeros((M,), np.float32)
        ones[:ml] = 1.0
        vones_c = np.ascontiguousarray(
            np.repeat(ones.reshape(MT, P).T[:, :, None], HL, axis=2).reshape(
                P, MT * HL
            )
        ).astype(bf)
        in_maps.append(
            {
                "xT": xT_c,
                "wqkpq": wqkpq_c,
                "wqkpk": wqkpk_c,
                "wqkr": wqkr_c,
                "wv": wv_c,
                "wout": wout_c,
                "vones": vones_c,
            }
        )
    return in_maps, live, M


def kernel(x, w_qkv, w_out, b_out, mask):
    from concourse.bass_utils import run_bass_kernel_spmd

    x = np.asarray(x, dtype=np.float32)
    w_qkv = np.asarray(w_qkv, dtype=np.float32)
    w_out_f = np.asarray(w_out, dtype=np.float32)
    b_out = np.asarray(b_out, dtype=np.float32)
    mask = np.asarray(mask)

    in_maps, live, M = _shard_inputs(x, w_qkv, w_out_f, b_out, mask)
    meff = -(-max(1, max(len(l) for l in live)) // 32) * 32
    key = (M, meff)
    if key not in _CACHE:
        _CACHE[key] = _build_program(M, meff)
    nc = _CACHE[key]

    res = run_bass_kernel_spmd(nc, in_maps, list(range(NCORES))).results

    MT = M // P
    IT = min(MT, 8)
    IEFF = IT * P
    w3d = w_qkv.astype(np.float64).reshape(DM, H, 3, DH)
    wv_full = w3d[:, :, 2, :].reshape(DM, H * DH)
    w3 = w_qkv.reshape(DM, H, 3, DH)
    out = np.empty((B, N, DM), np.float32)
    for b in range(B):
        idx = live[b]
        ml = len(idx)
        mldev = min(ml, IEFF)
        part = res[HG * b]["out"].astype(np.float32)[:mldev] + res[HG * b + 1][
            "out"
        ].astype(np.float32)[:mldev]
        # dead-query rows: uniform softmax over ALL tokens -> mean of v
        # rows through the output projection; exact float64 on the host.
        dead_row = (
            x[b].astype(np.float64).mean(axis=0) @ wv_full @ w_out_f.astype(np.float64)
        ).astype(np.float32)
        out[b] = dead_row[None, :] + b_out[None, :]
        out[b][idx[:mldev]] = part + b_out[None, :]
        if ml > IEFF:
            # overflow live queries (tokens beyond the device i-range):
            # exact attention on the host from the device-exported k / v
            # (bf16, so identical rounding to the device path).
            rows = idx[IEFF:ml]
            xq = x[b][rows]
            acc = np.empty((len(rows), H * DH), np.float32)
            for hg in range(HG):
                c = HG * b + hg
                kxa = res[c]["kx"].astype(np.float32)
                vpa = res[c]["vpx"].astype(np.float32)
                k_loc = (
                    kxa.reshape(P, NP, M).transpose(2, 1, 0).reshape(M, HL * DH)
                )
                v_loc = (
                    vpa.reshape(P, MT, HL, VW)[:, :, :, :DH]
                    .transpose(1, 0, 2, 3)
                    .reshape(M, HL, DH)
                )
                wq = w3[:, hg * HL : (hg + 1) * HL, 0, :].reshape(DM, HL * DH)
                q = (xq @ wq).reshape(-1, HL, DH) * SCALE
                for lh in range(HL):
                    s = q[:, lh] @ k_loc[:ml, lh * DH : (lh + 1) * DH].T
                    t = np.exp(s)
                    o = (t @ v_loc[:ml, lh]) / t.sum(axis=1)[:, None]
                    acc[:, (hg * HL + lh) * DH : (hg * HL + lh + 1) * DH] = o
            out[b][rows] = acc @ w_out_f + b_out[None, :]
    return out
